# revision 15
# baseline (speedup 1.0000x reference)
"""Trainium2 Bass kernel for the CPC contrastive loss problem.

Math (reference):
    fx = relu(x @ W1 + b1) @ W2 + b2          [N, Z]
    fz = z @ Wz + bz                          [N, Z]
    u[n] = fx[n] @ Ws[c[n]]                   [N, Z]
    T = softplus(<u, fz>_row)                 [N]
    neg_T[i] = mean_{j: c[j]==c[i]} softplus(<u[i], fz[j]>)
    out = log(T + eps) - log(neg_T + eps)

Structure: rows are grouped by category on the host; each of the 8 cores gets
8 categories, so the NxN S matrix reduces to per-category blocks (64x less
work). Categories are rank-sorted by size; slot s holds same-rank categories
on every core, so the slot widths W[s] (max size in the rank group) bake into
one SPMD program. Slot positions interleave large/small ranks so adjacent
pairs (the processing blocks) are >= 256 columns wide: fp32r matmuls below
256 output columns run at 1/4 rate.

Key optimizations vs a straight port:
  - x and z ship as bf16 hi+lo pairs (halves the dominant DMA volume); W1/Wz
    stay fp32r, so the product precision is unchanged (the hi+lo pair
    reconstructs the input to ~2^-17, and fp32r weight rounding dominates).
  - neg_T uses relu instead of softplus: S entries have std ~89, so the
    log1p(exp(-|S|)) correction inside a 100+-term mean inside a log is
    ~2e-5 relative (measured) vs the 2e-2 budget. This deletes the entire
    Abs/Exp/Ln/reduce elementwise tail over S.
  - One DMA per block (z and x halves stacked in one dram tensor): the cost
    model charges ~650ns of issue time per DMA, so few large transfers beat
    many small ones.
  - The device returns d = <u,fz> and q2 = mean_j relu(S) per row; the final
    log(softplus(d)+eps) - log(q2+eps) is O(N) scalar work done in float64
    on the host during unsharding (exact softplus, no LUT range issues).
  - u's bias is folded into the matmul via an all-ones row at partition HA
    of the h1 tile (engine partition starts must be multiples of 32).
  - PE work is software-pipelined one block behind the fz/h1 matmuls so the
    in-order PE queue never head-blocks waiting for the bf16 casts.
  - Elementwise work is spread over DVE/ACT/Pool; Pool reads PSUM fine.
"""

import sys

for _p in ("/opt/trn_rl_repo", "/root/.axon_site/_ro/trn_rl_repo"):
    if _p not in sys.path:
        sys.path.append(_p)

import numpy as np
import ml_dtypes

import concourse.bacc as bacc
import concourse.tile as tile
from concourse import mybir as mb
from concourse.bass_utils import run_bass_kernel_spmd

BF16NP = ml_dtypes.bfloat16

# ---------------------------------------------------------------- constants
N, IN, Z, C, H = 8192, 512, 128, 64, 50
NCORES = 8
G = C // NCORES          # category slots per core
KX = IN // 128           # k-tiles for x
KZ = 2 + 2 * KX          # bf16 row-groups in the xz tensor: zh zl xh*4 xl*4
EPS = 1e-8
N_WARM = 8
HA = 64                  # partition row holding the ones for the folded u bias

F = mb.ActivationFunctionType
OP = mb.AluOpType
FP32 = mb.dt.float32
FP32R = mb.dt.float32r
BF16 = mb.dt.bfloat16

_PROGRAMS = {}


class Layout:
    """Slot/chunk/block geometry baked into the program (shared by cores)."""

    def __init__(self, widths):
        assert len(widths) == G
        self.W = list(widths)
        self.OFF = np.concatenate([[0], np.cumsum(self.W)]).astype(int)
        self.R = int(self.OFF[-1])
        # chunks: (slot, coff, cw, ci)
        self.chunks = []
        for s, w in enumerate(self.W):
            for coff in range(0, w, 128):
                self.chunks.append((s, coff, min(128, w - coff), len(self.chunks)))
        self.NCHUNK = len(self.chunks)
        self.blocks = [(s, min(s + 2, G)) for s in range(0, G, 2)]
        # packA column layout
        self.PK_W1 = (0, KX * H)
        self.PK_WZ = (KX * H, KX * H + Z)
        o = KX * H + Z
        self.PK_PINV = (o, o + self.NCHUNK)
        o += self.NCHUNK
        self.PK_B1 = (o, o + 1)
        self.PW = o + 1
        self.PK_BZ = (0, Z)
        self.PK_MROW = (Z, Z + self.R)
        self.PW1 = Z + self.R

    def ok(self):
        return all(
            int(self.OFF[s1] - self.OFF[s0]) >= 256 for s0, s1 in self.blocks
        ) and max(self.W) <= 256

    def key(self):
        return tuple(self.W)


def _build_program(L: Layout):
    nc = bacc.Bacc("TRN2", target_bir_lowering=False, debug=False)

    R, NC_ = L.R, L.NCHUNK
    d_xz = nc.dram_tensor("xz", [KZ * 128, R], BF16, kind="ExternalInput").ap()
    d_packA = nc.dram_tensor("packA", [128, L.PW], FP32, kind="ExternalInput").ap()
    d_pack1 = nc.dram_tensor("pack1", [1, L.PW1], FP32, kind="ExternalInput").ap()
    d_w2s = nc.dram_tensor("w2s", [HA + 1, G * Z], FP32, kind="ExternalInput").ap()
    d_yout = nc.dram_tensor("yout", [128 * 2 * NC_], FP32, kind="ExternalOutput").ap()

    xz_view = d_xz.rearrange("(k p) n -> p k n", p=128)

    with tile.TileContext(nc) as tc:
        with (
            tc.tile_pool(name="const", bufs=1) as const,
            tc.tile_pool(name="junk", bufs=3) as junkp,
            tc.tile_pool(name="blk", bufs=2) as blkp,
            tc.tile_pool(name="psum_z", bufs=2, space="PSUM") as psum_z,
            tc.tile_pool(name="psum_h", bufs=2, space="PSUM") as psum_h,
            tc.tile_pool(name="psum_u", bufs=1, space="PSUM") as psum_u,
            tc.tile_pool(name="psum_s", bufs=1, space="PSUM") as psum_s,
            tc.tile_pool(name="psum_d", bufs=1, space="PSUM") as psum_d,
        ):
            # ---- constants
            s_ones = const.tile([128, 1], FP32)
            nc.vector.memset(s_ones[:], 1.0)
            # the one ACT table set (id 6) holding Copy/Relu used below
            nc.scalar.add_instruction(
                mb.InstLoadActFuncSet(
                    name=nc.get_next_instruction_name(),
                    ins=[],
                    outs=[],
                    act_func_set_id=6,
                )
            )
            s_warmact = const.tile([128, 1], FP32)
            nc.scalar.activation(out=s_warmact[:], in_=s_ones[:], func=F.Abs)

            # ---- persistent tiles
            s_xz = const.tile([128, KZ, R], BF16)
            s_h1T = const.tile([HA + 1, R], FP32R)
            s_fz16 = const.tile([128, R], BF16)
            s_out = const.tile([128, 2, NC_], FP32)  # [:,0,:] q2, [:,1,:] d
            s_packA = const.tile([128, L.PW], FP32R)
            s_pack1 = const.tile([1, L.PW1], FP32R)
            s_w2s = const.tile([HA + 1, G * Z], FP32R)

            # ---- all DMAs up front in issue order
            nc.sync.dma_start(out=s_packA[:], in_=d_packA.bitcast(FP32R)[:])
            nc.sync.dma_start(out=s_pack1[:], in_=d_pack1.bitcast(FP32R)[:])
            for bi, (s0, s1) in enumerate(L.blocks):
                ns = slice(int(L.OFF[s0]), int(L.OFF[s1]))
                nc.sync.dma_start(out=s_xz[:, :, ns], in_=xz_view[:, :, ns])
                if bi == 0:
                    nc.sync.dma_start(out=s_w2s[:], in_=d_w2s.bitcast(FP32R)[:])

            def pk(lo_hi, rows=128, cast=None):
                ap = s_packA[0:rows, lo_hi[0] : lo_hi[1]]
                return ap.bitcast(cast) if cast else ap

            s_w1 = pk(L.PK_W1).rearrange("p (k h) -> p k h", k=KX)
            s_wz = pk(L.PK_WZ)
            s_pinv = pk(L.PK_PINV, cast=FP32)
            s_b1 = pk(L.PK_B1, rows=H, cast=FP32)
            s_bz = s_pack1[0:1, L.PK_BZ[0] : L.PK_BZ[1]]
            s_mrow = s_pack1[0:1, L.PK_MROW[0] : L.PK_MROW[1]]

            # PE warm-up to start the p-state ramp while DMA runs
            pwarm = psum_z.tile([1, 64], FP32, tag="pz")
            s_wrhs = const.tile([128, 64], FP32)
            nc.vector.memset(s_wrhs[:], 0.0)
            for _ in range(N_WARM):
                nc.tensor.matmul(
                    pwarm[:], lhsT=s_ones[:], rhs=s_wrhs[:], start=True, stop=True
                )

            # ones row (partition HA) for the folded u bias: u = W2s_aug^T
            # [h1; ...; 1]. Rows H..HA zeroed (partition starts must be
            # multiples of 32; rows 32..H are overwritten by every h1 block).
            nc.vector.memset(s_h1T.bitcast(FP32)[32:HA, :], 0.0)
            nc.vector.memset(s_h1T.bitcast(FP32)[HA : HA + 1, :], 1.0)
            # chunks narrower than 128 leave tail partitions untouched
            nc.vector.memset(s_out[:], 0.0)
            pd = psum_d.tile([128, NC_], FP32)
            nc.vector.memset(pd[:], 0.0)

            state = {}

            def emit_uS(bi):
                """u matmuls, casts, prod/d, S and relu-accums for block bi."""
                s0, s1 = L.blocks[bi]
                boff = int(L.OFF[s0])
                bw = int(L.OFF[s1] - L.OFF[s0])
                pz = state[bi]["pz"]
                nsb = s1 - s0
                pu = psum_u.tile([128, nsb, 256], FP32, tag="pu")
                ush = []
                for j, s in enumerate(range(s0, s1)):
                    rhs_off = min(int(L.OFF[s]), boff + bw - 256)
                    ush.append(int(L.OFF[s]) - rhs_off)
                    nc.tensor.matmul(
                        pu[:, j, :],
                        lhsT=s_w2s[:, s * Z : (s + 1) * Z],
                        rhs=s_h1T[:, rhs_off : rhs_off + 256],
                        start=True,
                        stop=True,
                    )
                last = bi == len(L.blocks) - 1
                s_u16b = blkp.tile([128, nsb, 256], BF16, tag="u16")
                if not last:
                    nc.gpsimd.tensor_copy(s_u16b[:], pu[:])
                else:
                    for j in range(nsb):  # per-slot on ACT: shorter tail chain
                        nc.scalar.activation(
                            out=s_u16b[:, j, :], in_=pu[:, j, :], func=F.Copy
                        )

                bchunks = [ch for ch in L.chunks if s0 <= ch[0] < s1]
                pS = psum_s.tile([128, len(bchunks), 256], FP32, tag="ps")
                s_prodb = blkp.tile([128, nsb, 256], FP32, tag="prod")
                for j, s in enumerate(range(s0, s1)):
                    w = L.W[s]
                    soff = int(L.OFF[s])
                    nc.vector.tensor_mul(
                        s_prodb[:, j, ush[j] : ush[j] + w],
                        pu[:, j, ush[j] : ush[j] + w],
                        pz[:, soff - boff : soff - boff + w],
                    )
                    for (cs, coff, cw, ci) in bchunks:
                        if cs != s:
                            continue
                        nc.tensor.matmul(
                            pd[0:cw, ci : ci + 1],
                            lhsT=s_prodb[:, j, ush[j] + coff : ush[j] + coff + cw],
                            rhs=s_ones[:],
                            start=True,
                            stop=True,
                        )
                        ck = ci - bchunks[0][3]
                        nc.tensor.matmul(
                            pS[0:cw, ck, 0:w],
                            lhsT=s_u16b[:, j, ush[j] + coff : ush[j] + coff + cw],
                            rhs=s_fz16[:, soff : soff + w],
                            start=True,
                            stop=True,
                        )
                        # sum_j relu(S) straight from PSUM (accum_out's
                        # reduction op is op1, so it must stay add); the 1/n
                        # mean folds into the host-side final
                        jk = junkp.tile([128, 256], FP32, tag="junk")
                        acc = s_out[0:cw, 0, ci : ci + 1]
                        if ci % 2 == 0:
                            nc.vector.tensor_scalar(
                                out=jk[0:cw, 0:w], in0=pS[0:cw, ck, 0:w],
                                scalar1=0.0, scalar2=None, op0=OP.max, op1=OP.add,
                                accum_out=acc,
                            )
                        else:
                            nc.scalar.activation(
                                out=jk[0:cw, 0:w], in_=pS[0:cw, ck, 0:w],
                                func=F.Relu, accum_out=acc,
                            )

            for bi, (s0, s1) in enumerate(L.blocks):
                boff = int(L.OFF[s0])
                bw = int(L.OFF[s1] - L.OFF[s0])
                ns = slice(boff, boff + bw)

                # fz = Wz^T (zh + zl) + bz x mrow (rank-1 keeps pads zero)
                pz = psum_z.tile([128, bw], FP32, tag="pz")
                nc.tensor.matmul(
                    pz[:], lhsT=s_wz, rhs=s_xz[:, 0, ns], start=True, stop=False
                )
                nc.tensor.matmul(
                    pz[:], lhsT=s_wz, rhs=s_xz[:, 1, ns], start=False, stop=False
                )
                nc.tensor.matmul(
                    pz[:], lhsT=s_bz, rhs=s_mrow[:, ns], start=False, stop=True
                )
                nc.gpsimd.tensor_copy(s_fz16[:, ns], pz[:])
                state[bi] = {"pz": pz}

                # h1 = relu(W1^T (xh + xl) + b1)
                ph = psum_h.tile([H, bw], FP32, tag="ph")
                for kk in range(2 * KX):
                    nc.tensor.matmul(
                        ph[:],
                        lhsT=s_w1[:, kk % KX, :],
                        rhs=s_xz[:, 2 + kk, ns],
                        start=(kk == 0),
                        stop=(kk == 2 * KX - 1),
                    )
                nc.vector.tensor_scalar(
                    out=s_h1T.bitcast(FP32)[0:H, ns], in0=ph[:], scalar1=s_b1,
                    scalar2=0.0, op0=OP.add, op1=OP.max,
                )
                if bi > 0:
                    emit_uS(bi - 1)
            emit_uS(len(L.blocks) - 1)

            # d column straight out of PSUM into the output tile
            nc.vector.tensor_copy(s_out[:, 1, :], pd[:])
            nc.sync.dma_start(
                out=d_yout.rearrange("(p t c) -> p t c", p=128, t=2), in_=s_out[:]
            )

    nc.compile()
    return nc


def get_program(L: Layout):
    k = L.key()
    if k not in _PROGRAMS:
        _PROGRAMS[k] = _build_program(L)
    return _PROGRAMS[k]


# ---------------------------------------------------------------- host side
def _assign(cf):
    """Rank-sort categories; rank group g goes to slot position POS[g] so
    adjacent slot pairs (the matmul blocks) are >= 256 wide."""
    sizes = np.array([(cf == k).sum() for k in range(C)])
    order = np.argsort(-sizes, kind="stable")
    pos_of_group = [0, 2, 4, 6, 7, 5, 3, 1]
    widths = [0] * G
    catmap = [[0] * G for _ in range(NCORES)]
    nmap = [[0] * G for _ in range(NCORES)]
    for g in range(G):
        grp = order[8 * g : 8 * g + 8]
        p = pos_of_group[g]
        widths[p] = int(sizes[grp[0]])
        for core in range(NCORES):
            catmap[core][p] = int(grp[core])
            nmap[core][p] = int(sizes[grp[core]])
    return widths, catmap, nmap


def _hi_lo(a):
    hi = a.astype(BF16NP)
    lo = (a - hi.astype(np.float32)).astype(BF16NP)
    return hi, lo


def _prep_core_inputs(L, x, z, Ws, W1, b1, W2, b2, Wz, bz, idx_lists, catmap_c, nmap_c):
    xz = np.zeros((KZ * 128, L.R), BF16NP)
    pack1 = np.zeros((1, L.PW1), np.float32)
    pinv = np.ones((128, L.NCHUNK), np.float32)
    for s in range(G):
        idx = idx_lists[catmap_c[s]]
        n = nmap_c[s]
        lo = int(L.OFF[s])
        if n:
            zh, zl = _hi_lo(z[idx].T)
            xz[0:128, lo : lo + n] = zh
            xz[128:256, lo : lo + n] = zl
            xh, xl = _hi_lo(x[idx].T)
            xz[256 : 256 + IN, lo : lo + n] = xh
            xz[256 + IN :, lo : lo + n] = xl
            pack1[0, L.PK_MROW[0] + lo : L.PK_MROW[0] + lo + n] = 1.0
    for (s, coff, cw, ci) in L.chunks:
        pinv[:, ci] = 1.0 / max(nmap_c[s], 1)
    pack1[0, L.PK_BZ[0] : L.PK_BZ[1]] = bz

    packA = np.zeros((128, L.PW), np.float32)
    packA[:, L.PK_W1[0] : L.PK_W1[1]] = (
        W1.reshape(KX, 128, H).transpose(1, 0, 2).reshape(128, KX * H)
    )
    packA[:, L.PK_WZ[0] : L.PK_WZ[1]] = Wz
    packA[:, L.PK_PINV[0] : L.PK_PINV[1]] = pinv
    packA[:H, L.PK_B1[0]] = b1
    # fold the second MLP layer and its bias into each slot's bilinear weight
    w2s = np.zeros((HA + 1, G * Z), np.float32)
    for s in range(G):
        Wsg = Ws[catmap_c[s]].astype(np.float64)
        w2s[:H, s * Z : (s + 1) * Z] = (W2.astype(np.float64) @ Wsg).astype(np.float32)
        w2s[HA, s * Z : (s + 1) * Z] = (b2.astype(np.float64) @ Wsg).astype(np.float32)
    return {"xz": xz, "packA": packA, "pack1": pack1, "w2s": w2s}


def _unpack_core_output(L, y, idx_lists, catmap_c, nmap_c, out):
    """y flat [(p t c)] -> rows; final log(softplus(d)+eps)-log(q2+eps) in
    float64 on the host (O(N) unshard-time scalar work)."""
    y = np.asarray(y).reshape(128, 2, L.NCHUNK).astype(np.float64)
    rel = y[:, 0, :]
    d = y[:, 1, :]
    T = np.log1p(np.exp(-np.abs(d))) + np.maximum(d, 0.0)
    logT = np.log(T + EPS)
    for (s, coff, cw, ci) in L.chunks:
        n = nmap_c[s]
        take = min(cw, n - coff)
        if take > 0:
            idx = idx_lists[catmap_c[s]][coff : coff + take]
            out[idx] = logT[0:take, ci] - np.log(rel[0:take, ci] / n + EPS)


def _numpy_fallback(x, c, z, W1, b1, W2, b2, Wz, bz, Ws):
    x64 = x.astype(np.float64)
    fx = np.maximum(x64 @ W1.astype(np.float64) + b1, 0.0) @ W2.astype(
        np.float64
    ) + b2
    fz = z.astype(np.float64) @ Wz.astype(np.float64) + bz
    u = np.einsum("nd,nde->ne", fx, Ws.astype(np.float64)[c])

    def sp(v):
        return np.log1p(np.exp(-np.abs(v))) + np.maximum(v, 0.0)

    T = sp(np.einsum("ne,ne->n", u, fz))
    out = np.empty(N, np.float64)
    for k in range(C):
        idx = np.where(c == k)[0]
        if len(idx) == 0:
            continue
        Sk = sp(u[idx] @ fz[idx].T)
        out[idx] = np.log(T[idx] + EPS) - np.log(Sk.mean(axis=1) + EPS)
    return out.astype(np.float32)


def kernel(x, c, z, W1, b1, W2, b2, Wz, bz, Ws):
    x = np.ascontiguousarray(np.asarray(x), dtype=np.float32)
    z = np.ascontiguousarray(np.asarray(z), dtype=np.float32)
    W1 = np.ascontiguousarray(np.asarray(W1), dtype=np.float32)
    b1 = np.ascontiguousarray(np.asarray(b1), dtype=np.float32)
    W2 = np.ascontiguousarray(np.asarray(W2), dtype=np.float32)
    b2 = np.ascontiguousarray(np.asarray(b2), dtype=np.float32)
    Wz = np.ascontiguousarray(np.asarray(Wz), dtype=np.float32)
    bz = np.ascontiguousarray(np.asarray(bz), dtype=np.float32)
    Ws = np.ascontiguousarray(np.asarray(Ws), dtype=np.float32)
    cf = np.asarray(c).reshape(-1).astype(np.int64)

    idx_lists = [np.where(cf == k)[0] for k in range(C)]
    sizes = [len(i) for i in idx_lists]
    if max(sizes) > 256 or min(sizes) == 0 or len(cf) != N:
        return _numpy_fallback(x, cf, z, W1, b1, W2, b2, Wz, bz, Ws)

    widths, catmap, nmap = _assign(cf)
    L = Layout(widths)
    if not L.ok():
        return _numpy_fallback(x, cf, z, W1, b1, W2, b2, Wz, bz, Ws)

    in_maps = [
        _prep_core_inputs(
            L, x, z, Ws, W1, b1, W2, b2, Wz, bz, idx_lists, catmap[core], nmap[core]
        )
        for core in range(NCORES)
    ]

    nc = get_program(L)
    res = run_bass_kernel_spmd(nc, in_maps, core_ids=list(range(NCORES)))

    out = np.empty(N, np.float32)
    for core in range(NCORES):
        _unpack_core_output(
            L, res.results[core]["yout"], idx_lists, catmap[core], nmap[core], out
        )
    return out


# revision 16
# speedup vs baseline: 1.1118x; 1.1118x over previous
"""Trainium2 Bass kernel for the CPC contrastive loss problem.

Math (reference):
    fx = relu(x @ W1 + b1) @ W2 + b2          [N, Z]
    fz = z @ Wz + bz                          [N, Z]
    u[n] = fx[n] @ Ws[c[n]]                   [N, Z]
    T = softplus(<u, fz>_row)                 [N]
    neg_T[i] = mean_{j: c[j]==c[i]} softplus(<u[i], fz[j]>)
    out = log(T + eps) - log(neg_T + eps)

Structure: rows are grouped by category on the host; each of the 8 cores gets
8 categories, so the NxN S matrix reduces to per-category blocks (64x less
work). Categories are rank-sorted by size; slot s holds same-rank categories
on every core, so the slot widths W[s] (max size in the rank group) bake into
one SPMD program. Slot positions interleave large/small ranks so adjacent
pairs (the processing blocks) are >= 256 columns wide: fp32r matmuls below
256 output columns run at 1/4 rate.

Key optimizations:
  - x, W1, Wz ship/compute in fp16: fp16 has the same 10-bit mantissa as the
    fp32r (tf32-like) matmul mode used anyway, so accuracy is unchanged
    (measured 8e-4 end-to-end vs the 2e-2 budget) while x DMA halves. z rides
    along as an fp16 hi+lo pair (exact to fp32), since the d = <u,fz> diagonal
    needs full precision. b1/bz fold in via rank-1 matmuls against a
    valid-row mask (which also keeps padded columns exactly zero).
  - neg_T uses relu instead of softplus: S entries have std ~89, so the
    log1p(exp(-|S|)) correction inside a 100+-term mean inside a log is
    ~2e-5 relative. This deletes the whole Abs/Exp/Ln/reduce tail over S.
  - One DMA per block (z pair + x k-chunks stacked in one fp16 tensor): the
    cost model charges ~650ns issue + ~625ns HWDGE per DMA, so few large
    transfers win. The last block splits z/x so fz clears the tail early.
  - The device returns d and sum_j relu(S) per row; the final
    log(softplus(d)+eps) - log(mean+eps) is O(N) float64 work on the host
    during unsharding (exact softplus, no LUT range issues).
  - u's bias is folded via an all-ones row at partition HA of the h1 tile
    (engine partition starts must be multiples of 32).
  - PE work for block b-1's u/S stage is emitted before block b's fz/h1 so
    the in-order PE queue always has data-ready work in front.
  - Elementwise work spreads over DVE/ACT/Pool (Pool reads PSUM fine).
"""

import sys

for _p in ("/opt/trn_rl_repo", "/root/.axon_site/_ro/trn_rl_repo"):
    if _p not in sys.path:
        sys.path.append(_p)

import numpy as np

import concourse.bacc as bacc
import concourse.tile as tile
from concourse import mybir as mb
from concourse.bass_utils import run_bass_kernel_spmd

# ---------------------------------------------------------------- constants
N, IN, Z, C, H = 8192, 512, 128, 64, 50
NCORES = 8
G = C // NCORES          # category slots per core
KX = IN // 128           # k-tiles for x
KZ = 2 + KX              # fp16 row-groups in the xz tensor: zh zl x0..x3
EPS = 1e-8
N_WARM = 8
HA = 64                  # partition row holding the ones for the folded u bias

F = mb.ActivationFunctionType
OP = mb.AluOpType
FP32 = mb.dt.float32
FP32R = mb.dt.float32r
FP16 = mb.dt.float16
BF16 = mb.dt.bfloat16

_PROGRAMS = {}


class Layout:
    """Slot/chunk/block geometry baked into the program (shared by cores)."""

    def __init__(self, widths):
        assert len(widths) == G
        self.W = list(widths)
        self.OFF = np.concatenate([[0], np.cumsum(self.W)]).astype(int)
        self.R = int(self.OFF[-1])
        # chunks: (slot, coff, cw, ci)
        self.chunks = []
        for s, w in enumerate(self.W):
            for coff in range(0, w, 128):
                self.chunks.append((s, coff, min(128, w - coff), len(self.chunks)))
        self.NCHUNK = len(self.chunks)
        self.blocks = [(s, min(s + 2, G)) for s in range(0, G, 2)]
        # packA column layout (fp16; W1/Wz span all partitions, the rest are
        # row-0 vectors)
        self.PK_W1 = (0, KX * H)
        self.PK_WZ = (KX * H, KX * H + Z)
        o = KX * H + Z
        self.PK_B1R = (o, o + H)
        o += H
        self.PK_BZ = (o, o + Z)
        o += Z
        self.PK_MROW = (o, o + self.R)
        self.PW = o + self.R

    def ok(self):
        return all(
            int(self.OFF[s1] - self.OFF[s0]) >= 256 for s0, s1 in self.blocks
        ) and max(self.W) <= 256

    def key(self):
        return tuple(self.W)


def _build_program(L: Layout):
    nc = bacc.Bacc("TRN2", target_bir_lowering=False, debug=False)

    R, NC_ = L.R, L.NCHUNK
    d_xz = nc.dram_tensor("xz", [KZ * 128, R], FP16, kind="ExternalInput").ap()
    d_packA = nc.dram_tensor("packA", [128, L.PW], FP16, kind="ExternalInput").ap()
    d_w2s = nc.dram_tensor("w2s", [HA + 1, G * Z], FP32, kind="ExternalInput").ap()
    d_yout = nc.dram_tensor("yout", [128 * 2 * NC_], FP32, kind="ExternalOutput").ap()

    xz_view = d_xz.rearrange("(k p) n -> p k n", p=128)
    NB = len(L.blocks)

    with tile.TileContext(nc) as tc:
        with (
            tc.tile_pool(name="const", bufs=1) as const,
            tc.tile_pool(name="junk", bufs=3) as junkp,
            tc.tile_pool(name="blk", bufs=2) as blkp,
            tc.tile_pool(name="psum_z", bufs=2, space="PSUM") as psum_z,
            tc.tile_pool(name="psum_h", bufs=2, space="PSUM") as psum_h,
            tc.tile_pool(name="psum_u", bufs=1, space="PSUM") as psum_u,
            tc.tile_pool(name="psum_s", bufs=1, space="PSUM") as psum_s,
            tc.tile_pool(name="psum_d", bufs=1, space="PSUM") as psum_d,
        ):
            # ---- constants
            s_ones = const.tile([128, 1], FP32)
            nc.vector.memset(s_ones[:], 1.0)
            # the one ACT table set (id 6) holding Copy/Relu used below
            nc.scalar.add_instruction(
                mb.InstLoadActFuncSet(
                    name=nc.get_next_instruction_name(),
                    ins=[],
                    outs=[],
                    act_func_set_id=6,
                )
            )
            s_warmact = const.tile([128, 1], FP32)
            nc.scalar.activation(out=s_warmact[:], in_=s_ones[:], func=F.Abs)

            # ---- persistent tiles
            s_xz = const.tile([128, KZ, R], FP16)
            s_h1T = const.tile([HA + 1, R], FP32R)
            s_fz16 = const.tile([128, R], BF16)
            s_out = const.tile([128, 2, NC_], FP32)  # [:,0,:] relu-sums, [:,1,:] d
            s_packA = const.tile([128, L.PW], FP16)
            s_w2s = const.tile([HA + 1, G * Z], FP32R)

            # ---- all DMAs up front in issue order
            nc.sync.dma_start(out=s_packA[:], in_=d_packA[:])
            for bi, (s0, s1) in enumerate(L.blocks):
                ns = slice(int(L.OFF[s0]), int(L.OFF[s1]))
                if bi == NB - 1:
                    # split z/x so the tail block's fz path clears early
                    nc.sync.dma_start(out=s_xz[:, 0:2, ns], in_=xz_view[:, 0:2, ns])
                    nc.sync.dma_start(out=s_xz[:, 2:KZ, ns], in_=xz_view[:, 2:KZ, ns])
                else:
                    nc.sync.dma_start(out=s_xz[:, :, ns], in_=xz_view[:, :, ns])
                if bi == 0:
                    nc.sync.dma_start(out=s_w2s[:], in_=d_w2s.bitcast(FP32R)[:])

            s_w1 = s_packA[:, L.PK_W1[0] : L.PK_W1[1]].rearrange(
                "p (k h) -> p k h", k=KX
            )
            s_wz = s_packA[:, L.PK_WZ[0] : L.PK_WZ[1]]
            s_b1r = s_packA[0:1, L.PK_B1R[0] : L.PK_B1R[1]]
            s_bz = s_packA[0:1, L.PK_BZ[0] : L.PK_BZ[1]]
            s_mrow = s_packA[0:1, L.PK_MROW[0] : L.PK_MROW[1]]

            # PE warm-up to start the p-state ramp while DMA runs
            pwarm = psum_z.tile([1, 64], FP32, tag="pz")
            s_wrhs = const.tile([128, 64], FP32)
            nc.vector.memset(s_wrhs[:], 0.0)
            for _ in range(N_WARM):
                nc.tensor.matmul(
                    pwarm[:], lhsT=s_ones[:], rhs=s_wrhs[:], start=True, stop=True
                )

            # ones row (partition HA) for the folded u bias: u = W2s_aug^T
            # [h1; ...; 1]. Rows H..HA zeroed (partition starts must be
            # multiples of 32; rows 32..H are overwritten by every h1 block).
            nc.vector.memset(s_h1T.bitcast(FP32)[32:HA, :], 0.0)
            nc.vector.memset(s_h1T.bitcast(FP32)[HA : HA + 1, :], 1.0)
            # chunks narrower than 128 leave tail partitions untouched
            nc.vector.memset(s_out[:], 0.0)
            pd = psum_d.tile([128, NC_], FP32)
            nc.vector.memset(pd[:], 0.0)

            state = {}

            def emit_uS(bi):
                """u matmuls, casts, prod/d, S and relu-accums for block bi."""
                s0, s1 = L.blocks[bi]
                boff = int(L.OFF[s0])
                bw = int(L.OFF[s1] - L.OFF[s0])
                pz = state[bi]["pz"]
                nsb = s1 - s0
                last = bi == NB - 1
                pu = psum_u.tile([128, nsb, 256], FP32, tag="pu")
                ush = []
                for j, s in enumerate(range(s0, s1)):
                    rhs_off = min(int(L.OFF[s]), boff + bw - 256)
                    ush.append(int(L.OFF[s]) - rhs_off)
                    nc.tensor.matmul(
                        pu[:, j, :],
                        lhsT=s_w2s[:, s * Z : (s + 1) * Z],
                        rhs=s_h1T[:, rhs_off : rhs_off + 256],
                        start=True,
                        stop=True,
                    )
                s_u16b = blkp.tile([128, nsb, 256], BF16, tag="u16")
                if not last:
                    nc.gpsimd.tensor_copy(s_u16b[:], pu[:])
                else:
                    for j in range(nsb):  # per-slot on ACT: shorter tail chain
                        nc.scalar.activation(
                            out=s_u16b[:, j, :], in_=pu[:, j, :], func=F.Copy
                        )

                bchunks = [ch for ch in L.chunks if s0 <= ch[0] < s1]
                pS = psum_s.tile([128, len(bchunks), 256], FP32, tag="ps")
                s_prodb = blkp.tile([128, nsb, 256], FP32, tag="prod")
                for j, s in enumerate(range(s0, s1)):
                    w = L.W[s]
                    soff = int(L.OFF[s])
                    nc.vector.tensor_mul(
                        s_prodb[:, j, ush[j] : ush[j] + w],
                        pu[:, j, ush[j] : ush[j] + w],
                        pz[:, soff - boff : soff - boff + w],
                    )
                    for (cs, coff, cw, ci) in bchunks:
                        if cs != s:
                            continue
                        nc.tensor.matmul(
                            pd[0:cw, ci : ci + 1],
                            lhsT=s_prodb[:, j, ush[j] + coff : ush[j] + coff + cw],
                            rhs=s_ones[:],
                            start=True,
                            stop=True,
                        )
                        ck = ci - bchunks[0][3]
                        nc.tensor.matmul(
                            pS[0:cw, ck, 0:w],
                            lhsT=s_u16b[:, j, ush[j] + coff : ush[j] + coff + cw],
                            rhs=s_fz16[:, soff : soff + w],
                            start=True,
                            stop=True,
                        )
                        # sum_j relu(S) straight from PSUM (accum_out's
                        # reduction op is op1 -> must be add); the 1/n mean
                        # folds into the host-side final
                        jk = junkp.tile([128, 256], FP32, tag="junk")
                        acc = s_out[0:cw, 0, ci : ci + 1]
                        if ck == 1:
                            nc.scalar.activation(
                                out=jk[0:cw, 0:w], in_=pS[0:cw, ck, 0:w],
                                func=F.Relu, accum_out=acc,
                            )
                        else:
                            nc.vector.tensor_scalar(
                                out=jk[0:cw, 0:w], in0=pS[0:cw, ck, 0:w],
                                scalar1=0.0, scalar2=None, op0=OP.max, op1=OP.add,
                                accum_out=acc,
                            )

            for bi, (s0, s1) in enumerate(L.blocks):
                if bi > 0:
                    emit_uS(bi - 1)
                boff = int(L.OFF[s0])
                bw = int(L.OFF[s1] - L.OFF[s0])
                ns = slice(boff, boff + bw)

                # fz = Wz^T (zh + zl) + bz x mrow (rank-1 keeps pads zero)
                pz = psum_z.tile([128, bw], FP32, tag="pz")
                nc.tensor.matmul(
                    pz[:], lhsT=s_wz, rhs=s_xz[:, 0, ns], start=True, stop=False
                )
                nc.tensor.matmul(
                    pz[:], lhsT=s_wz, rhs=s_xz[:, 1, ns], start=False, stop=False
                )
                nc.tensor.matmul(
                    pz[:], lhsT=s_bz, rhs=s_mrow[:, ns], start=False, stop=True
                )
                nc.scalar.activation(out=s_fz16[:, ns], in_=pz[:], func=F.Copy)
                state[bi] = {"pz": pz}

                # h1 = relu(W1^T x + b1 x mrow)
                ph = psum_h.tile([H, bw], FP32, tag="ph")
                for k in range(KX):
                    nc.tensor.matmul(
                        ph[:],
                        lhsT=s_w1[:, k, :],
                        rhs=s_xz[:, 2 + k, ns],
                        start=(k == 0),
                        stop=False,
                    )
                nc.tensor.matmul(
                    ph[:], lhsT=s_b1r, rhs=s_mrow[:, ns], start=False, stop=True
                )
                nc.scalar.activation(
                    out=s_h1T.bitcast(FP32)[0:H, ns], in_=ph[:], func=F.Relu
                )
            emit_uS(NB - 1)

            # d column straight out of PSUM into the output tile
            nc.vector.tensor_copy(s_out[:, 1, :], pd[:])
            nc.sync.dma_start(
                out=d_yout.rearrange("(p t c) -> p t c", p=128, t=2), in_=s_out[:]
            )

    nc.compile()
    return nc


def get_program(L: Layout):
    k = L.key()
    if k not in _PROGRAMS:
        _PROGRAMS[k] = _build_program(L)
    return _PROGRAMS[k]


# ---------------------------------------------------------------- host side
def _assign(cf):
    """Rank-sort categories; rank group g goes to slot position POS[g] so
    adjacent slot pairs (the matmul blocks) are >= 256 wide."""
    sizes = np.array([(cf == k).sum() for k in range(C)])
    order = np.argsort(-sizes, kind="stable")
    pos_of_group = [0, 2, 4, 6, 7, 5, 3, 1]
    widths = [0] * G
    catmap = [[0] * G for _ in range(NCORES)]
    nmap = [[0] * G for _ in range(NCORES)]
    for g in range(G):
        grp = order[8 * g : 8 * g + 8]
        p = pos_of_group[g]
        widths[p] = int(sizes[grp[0]])
        for core in range(NCORES):
            catmap[core][p] = int(grp[core])
            nmap[core][p] = int(sizes[grp[core]])
    return widths, catmap, nmap


def _prep_core_inputs(L, x, z, Ws, W1, b1, W2, b2, Wz, bz, idx_lists, catmap_c, nmap_c):
    xz = np.zeros((KZ * 128, L.R), np.float16)
    for s in range(G):
        idx = idx_lists[catmap_c[s]]
        n = nmap_c[s]
        lo = int(L.OFF[s])
        if n:
            zT = z[idx].T
            zh = zT.astype(np.float16)
            xz[0:128, lo : lo + n] = zh
            xz[128:256, lo : lo + n] = (zT - zh.astype(np.float32)).astype(np.float16)
            xz[256:, lo : lo + n] = x[idx].T.astype(np.float16)

    packA = np.zeros((128, L.PW), np.float16)
    packA[:, L.PK_W1[0] : L.PK_W1[1]] = (
        W1.reshape(KX, 128, H).transpose(1, 0, 2).reshape(128, KX * H)
    ).astype(np.float16)
    packA[:, L.PK_WZ[0] : L.PK_WZ[1]] = Wz.astype(np.float16)
    packA[0, L.PK_B1R[0] : L.PK_B1R[1]] = b1.astype(np.float16)
    packA[0, L.PK_BZ[0] : L.PK_BZ[1]] = bz.astype(np.float16)
    for s in range(G):
        lo = int(L.OFF[s])
        packA[0, L.PK_MROW[0] + lo : L.PK_MROW[0] + lo + nmap_c[s]] = 1.0

    # fold the second MLP layer and its bias into each slot's bilinear weight
    w2s = np.zeros((HA + 1, G * Z), np.float32)
    for s in range(G):
        Wsg = Ws[catmap_c[s]].astype(np.float64)
        w2s[:H, s * Z : (s + 1) * Z] = (W2.astype(np.float64) @ Wsg).astype(np.float32)
        w2s[HA, s * Z : (s + 1) * Z] = (b2.astype(np.float64) @ Wsg).astype(np.float32)
    return {"xz": xz, "packA": packA, "w2s": w2s}


def _unpack_core_output(L, y, idx_lists, catmap_c, nmap_c, out):
    """y flat [(p t c)] -> rows; final log(softplus(d)+eps)-log(mean+eps) in
    float64 on the host (O(N) unshard-time scalar work)."""
    y = np.asarray(y).reshape(128, 2, L.NCHUNK).astype(np.float64)
    rel = y[:, 0, :]
    d = y[:, 1, :]
    T = np.log1p(np.exp(-np.abs(d))) + np.maximum(d, 0.0)
    logT = np.log(T + EPS)
    for (s, coff, cw, ci) in L.chunks:
        n = nmap_c[s]
        take = min(cw, n - coff)
        if take > 0:
            idx = idx_lists[catmap_c[s]][coff : coff + take]
            out[idx] = logT[0:take, ci] - np.log(rel[0:take, ci] / n + EPS)


def _numpy_fallback(x, c, z, W1, b1, W2, b2, Wz, bz, Ws):
    x64 = x.astype(np.float64)
    fx = np.maximum(x64 @ W1.astype(np.float64) + b1, 0.0) @ W2.astype(
        np.float64
    ) + b2
    fz = z.astype(np.float64) @ Wz.astype(np.float64) + bz
    u = np.einsum("nd,nde->ne", fx, Ws.astype(np.float64)[c])

    def sp(v):
        return np.log1p(np.exp(-np.abs(v))) + np.maximum(v, 0.0)

    T = sp(np.einsum("ne,ne->n", u, fz))
    out = np.empty(N, np.float64)
    for k in range(C):
        idx = np.where(c == k)[0]
        if len(idx) == 0:
            continue
        Sk = sp(u[idx] @ fz[idx].T)
        out[idx] = np.log(T[idx] + EPS) - np.log(Sk.mean(axis=1) + EPS)
    return out.astype(np.float32)


def kernel(x, c, z, W1, b1, W2, b2, Wz, bz, Ws):
    x = np.ascontiguousarray(np.asarray(x), dtype=np.float32)
    z = np.ascontiguousarray(np.asarray(z), dtype=np.float32)
    W1 = np.ascontiguousarray(np.asarray(W1), dtype=np.float32)
    b1 = np.ascontiguousarray(np.asarray(b1), dtype=np.float32)
    W2 = np.ascontiguousarray(np.asarray(W2), dtype=np.float32)
    b2 = np.ascontiguousarray(np.asarray(b2), dtype=np.float32)
    Wz = np.ascontiguousarray(np.asarray(Wz), dtype=np.float32)
    bz = np.ascontiguousarray(np.asarray(bz), dtype=np.float32)
    Ws = np.ascontiguousarray(np.asarray(Ws), dtype=np.float32)
    cf = np.asarray(c).reshape(-1).astype(np.int64)

    idx_lists = [np.where(cf == k)[0] for k in range(C)]
    sizes = [len(i) for i in idx_lists]
    if max(sizes) > 256 or min(sizes) == 0 or len(cf) != N:
        return _numpy_fallback(x, cf, z, W1, b1, W2, b2, Wz, bz, Ws)

    widths, catmap, nmap = _assign(cf)
    L = Layout(widths)
    if not L.ok():
        return _numpy_fallback(x, cf, z, W1, b1, W2, b2, Wz, bz, Ws)

    in_maps = [
        _prep_core_inputs(
            L, x, z, Ws, W1, b1, W2, b2, Wz, bz, idx_lists, catmap[core], nmap[core]
        )
        for core in range(NCORES)
    ]

    nc = get_program(L)
    res = run_bass_kernel_spmd(nc, in_maps, core_ids=list(range(NCORES)))

    out = np.empty(N, np.float32)
    for core in range(NCORES):
        _unpack_core_output(
            L, res.results[core]["yout"], idx_lists, catmap[core], nmap[core], out
        )
    return out


# revision 22
# speedup vs baseline: 1.1776x; 1.0592x over previous
"""Trainium2 Bass kernel for the CPC contrastive loss problem.

Math (reference):
    fx = relu(x @ W1 + b1) @ W2 + b2          [N, Z]
    fz = z @ Wz + bz                          [N, Z]
    u[n] = fx[n] @ Ws[c[n]]                   [N, Z]
    T = softplus(<u, fz>_row)                 [N]
    neg_T[i] = mean_{j: c[j]==c[i]} softplus(<u[i], fz[j]>)
    out = log(T + eps) - log(neg_T + eps)

Structure: rows are grouped by category on the host; each of the 8 cores gets
8 categories, so the NxN S matrix reduces to per-category blocks (64x less
work). Categories are rank-sorted by size; slot s holds same-rank categories
on every core, so the slot widths W[s] (max size in the rank group) bake into
one SPMD program. Slot positions interleave large/small ranks so adjacent
pairs (the processing blocks) are >= 256 columns wide: fp32r matmuls below
256 output columns run at 1/4 rate.

Key optimizations:
  - Associativity: S = h1aug^T (W2s_aug fz) and d = colsum(h1aug * v) with
    v = W2s_aug fz, where W2s_aug = [W2 Ws[g]; b2 Ws[g]] folds the second MLP
    layer and its bias. This removes the whole u stage: the S matmuls take h1
    (already in SBUF after the relu) as the stationary side, and v depends
    only on z, which lands before x. Both blocks' v weights batch into one
    [128, 115]-wide stationary operand (slot 1's rows at partition 64 to
    respect the multiple-of-32 partition-start rule), so the v matmul runs at
    full rate over the >=256-wide block.
  - x, W1, Wz, W2s ship/compute in fp16: same 10-bit mantissa as the fp32r
    (tf32-like) matmul mode, so accuracy is unchanged (~7e-4 measured vs the
    2e-2 budget) while x DMA halves. z rides as an fp16 hi+lo pair (exact).
    b1/bz fold in via rank-1 matmuls against the valid-row mask, which also
    keeps padded columns exactly zero.
  - neg_T uses relu instead of softplus: S entries have std ~89, so the
    log1p(exp(-|S|)) correction inside a 100+-term mean inside a log is
    ~2e-5 relative. This deletes the whole Abs/Exp/Ln/reduce tail over S.
  - One DMA per block (z pair + x k-chunks stacked in one fp16 tensor): the
    cost model charges ~650ns issue + ~625ns HWDGE per DMA, so few large
    transfers win. The last block splits z/x so its v chain clears early.
  - The device returns d and sum_j relu(S) per row; the final
    log(softplus(d)+eps) - log(mean+eps) is O(N) float64 work on the host
    during unsharding (exact softplus, no LUT range issues).
  - Elementwise work spreads over DVE/ACT/Pool (Pool reads PSUM fine).
"""

import sys

for _p in ("/opt/trn_rl_repo", "/root/.axon_site/_ro/trn_rl_repo"):
    if _p not in sys.path:
        sys.path.append(_p)

import numpy as np

import concourse.bacc as bacc
import concourse.tile as tile
from concourse import mybir as mb
from concourse.bass_utils import run_bass_kernel_spmd

# ---------------------------------------------------------------- constants
N, IN, Z, C, H = 8192, 512, 128, 64, 50
NCORES = 8
G = C // NCORES          # category slots per core
KX = IN // 128           # k-tiles for x
KZ = 2 + KX              # fp16 row-groups in the xz tensor: zh zl x0..x3
EPS = 1e-8
N_WARM = 8
HB = H + 1               # 51: h1 rows plus the ones row at partition H
VP1 = 64                 # partition base of slot-1's v rows (multiple of 32)

F = mb.ActivationFunctionType
OP = mb.AluOpType
FP32 = mb.dt.float32
FP32R = mb.dt.float32r
FP16 = mb.dt.float16
BF16 = mb.dt.bfloat16

_PROGRAMS = {}


class Layout:
    """Slot/chunk/block geometry baked into the program (shared by cores)."""

    def __init__(self, widths):
        assert len(widths) == G
        self.W = list(widths)
        self.OFF = np.concatenate([[0], np.cumsum(self.W)]).astype(int)
        self.R = int(self.OFF[-1])
        # chunks: (slot, coff, cw, ci)
        self.chunks = []
        for s, w in enumerate(self.W):
            for coff in range(0, w, 128):
                self.chunks.append((s, coff, min(128, w - coff), len(self.chunks)))
        self.NCHUNK = len(self.chunks)
        self.blocks = [(s, min(s + 2, G)) for s in range(0, G, 2)]
        # packA column layout (fp16; W1/Wz/w2sT span partitions, the rest are
        # row-0 vectors)
        self.PK_W1 = (0, KX * H)
        o = KX * H
        self.PK_WZ = (o, o + Z)
        o += Z
        self.PK_W2T = (o, o + len(self.blocks) * (VP1 + HB))  # unused tail rows 0
        o += len(self.blocks) * (VP1 + HB)
        self.PK_B1R = (o, o + H)
        o += H
        self.PK_BZ = (o, o + Z)
        o += Z
        self.PK_MROW = (o, o + self.R)
        self.PW = o + self.R

    def ok(self):
        return all(
            int(self.OFF[s1] - self.OFF[s0]) >= 256 for s0, s1 in self.blocks
        ) and max(self.W) <= 170 and all(s1 - s0 == 2 for s0, s1 in self.blocks)

    def key(self):
        return tuple(self.W)


def _build_program(L: Layout):
    nc = bacc.Bacc("TRN2", target_bir_lowering=False, debug=False)

    R, NC_ = L.R, L.NCHUNK
    d_xz = nc.dram_tensor("xz", [KZ * 128, R], FP16, kind="ExternalInput").ap()
    d_packA = nc.dram_tensor("packA", [128, L.PW], FP16, kind="ExternalInput").ap()
    d_hones = nc.dram_tensor("hones", [1, R], FP32, kind="ExternalInput").ap()
    d_yout = nc.dram_tensor("yout", [128 * 2 * NC_], FP32, kind="ExternalOutput").ap()

    xz_view = d_xz.rearrange("(k p) n -> p k n", p=128)
    NB = len(L.blocks)

    with tile.TileContext(nc) as tc:
        with (
            tc.tile_pool(name="const", bufs=1) as const,
            tc.tile_pool(name="junk", bufs=3) as junkp,
            tc.tile_pool(name="psum_z", bufs=2, space="PSUM") as psum_z,
            tc.tile_pool(name="psum_h", bufs=2, space="PSUM") as psum_h,
            tc.tile_pool(name="psum_v", bufs=2, space="PSUM") as psum_v,
            tc.tile_pool(name="psum_s", bufs=1, space="PSUM") as psum_s,
            tc.tile_pool(name="psum_d", bufs=1, space="PSUM") as psum_d,
        ):
            # ---- constants
            s_ones = const.tile([128, 1], FP32)
            nc.vector.memset(s_ones[:], 1.0)
            # the one ACT table set (id 6) holding Copy/Relu used below
            nc.scalar.add_instruction(
                mb.InstLoadActFuncSet(
                    name=nc.get_next_instruction_name(),
                    ins=[],
                    outs=[],
                    act_func_set_id=6,
                )
            )
            s_warmact = const.tile([128, 1], FP32)
            nc.scalar.activation(out=s_warmact[:], in_=s_ones[:], func=F.Abs)

            # ---- persistent tiles
            s_xz = const.tile([128, KZ, R], FP16)
            s_h1T = const.tile([HB, R], FP32R)
            s_fz16 = const.tile([128, R], FP16)
            s_fz32 = const.tile([128, R], FP32R)
            s_v16 = const.tile([HB, R], FP16)
            s_out = const.tile([128, 2, NC_], FP32)  # [:,0,:] relu-sums, [:,1,:] d
            s_packA = const.tile([128, L.PW], FP16)

            # ---- all DMAs up front in issue order
            nc.sync.dma_start(out=s_packA[:], in_=d_packA[:])
            nc.sync.dma_start(out=s_h1T.bitcast(FP32)[H : H + 1, :], in_=d_hones[:])
            for bi, (s0, s1) in enumerate(L.blocks):
                ns = slice(int(L.OFF[s0]), int(L.OFF[s1]))
                if bi == NB - 1:
                    # split z/x so the tail block's v chain clears early
                    nc.sync.dma_start(out=s_xz[:, 0:2, ns], in_=xz_view[:, 0:2, ns])
                    nc.sync.dma_start(out=s_xz[:, 2:KZ, ns], in_=xz_view[:, 2:KZ, ns])
                else:
                    nc.sync.dma_start(out=s_xz[:, :, ns], in_=xz_view[:, :, ns])

            s_w1 = s_packA[:, L.PK_W1[0] : L.PK_W1[1]].rearrange(
                "p (k h) -> p k h", k=KX
            )
            s_wz = s_packA[:, L.PK_WZ[0] : L.PK_WZ[1]]
            s_w2T = s_packA[:, L.PK_W2T[0] : L.PK_W2T[1]].rearrange(
                "p (b q) -> p b q", b=NB
            )
            s_b1r = s_packA[0:1, L.PK_B1R[0] : L.PK_B1R[1]]
            s_bz = s_packA[0:1, L.PK_BZ[0] : L.PK_BZ[1]]
            s_mrow = s_packA[0:1, L.PK_MROW[0] : L.PK_MROW[1]]

            # PE warm-up to start the p-state ramp while DMA runs
            pwarm = psum_z.tile([1, 64], FP32, tag="pz")
            s_wrhs = const.tile([128, 64], FP32)
            nc.vector.memset(s_wrhs[:], 0.0)
            for _ in range(N_WARM):
                nc.tensor.matmul(
                    pwarm[:], lhsT=s_ones[:], rhs=s_wrhs[:], start=True, stop=True
                )

            # chunks narrower than 128 leave tail partitions untouched
            nc.vector.memset(s_out[:], 0.0)
            pd = psum_d.tile([128, NC_], FP32)
            nc.vector.memset(pd[:], 0.0)

            for bi, (s0, s1) in enumerate(L.blocks):
                boff = int(L.OFF[s0])
                bw = int(L.OFF[s1] - L.OFF[s0])
                ns = slice(boff, boff + bw)

                # fz = Wz^T (zh + zl) + bz x mrow (rank-1 keeps pads zero)
                pz = psum_z.tile([128, bw], FP32, tag="pz")
                nc.tensor.matmul(
                    pz[:], lhsT=s_wz, rhs=s_xz[:, 0, ns], start=True, stop=False
                )
                nc.tensor.matmul(
                    pz[:], lhsT=s_wz, rhs=s_xz[:, 1, ns], start=False, stop=False
                )
                nc.tensor.matmul(
                    pz[:], lhsT=s_bz, rhs=s_mrow[:, ns], start=False, stop=True
                )
                nc.scalar.activation(
                    out=s_fz32.bitcast(FP32)[:, ns], in_=pz[:], func=F.Copy
                )
                nc.gpsimd.tensor_copy(s_fz16[:, ns], pz[:])

                # h1 = relu(W1^T x + b1 x mrow)
                ph = psum_h.tile([H, bw], FP32, tag="ph")
                for k in range(KX):
                    nc.tensor.matmul(
                        ph[:],
                        lhsT=s_w1[:, k, :],
                        rhs=s_xz[:, 2 + k, ns],
                        start=(k == 0),
                        stop=False,
                    )
                nc.tensor.matmul(
                    ph[:], lhsT=s_b1r, rhs=s_mrow[:, ns], start=False, stop=True
                )
                nc.scalar.activation(
                    out=s_h1T.bitcast(FP32)[0:H, ns], in_=ph[:], func=F.Relu
                )

                # v = W2s_aug fz for both slots at once: rows 0:51 slot s0,
                # rows 64:115 slot s1 (partition starts must be 32-aligned)
                pv = psum_v.tile([VP1 + HB, bw], FP32, tag="pv")
                nc.tensor.matmul(
                    pv[:], lhsT=s_w2T[:, bi, :], rhs=s_fz32[:, ns],
                    start=True, stop=True,
                )
                # per-slot copies shift slot 1's rows down to partition base 0
                # (matmul needs lhsT/rhs bases to match)
                for j, s in enumerate(range(s0, s1)):
                    w = L.W[s]
                    so = int(L.OFF[s]) - boff
                    nc.gpsimd.tensor_copy(
                        s_v16[0:HB, boff + so : boff + so + w],
                        pv[VP1 * j : VP1 * j + HB, so : so + w],
                    )

                # S chunks, d columns, relu row-sums
                bchunks = [ch for ch in L.chunks if s0 <= ch[0] < s1]
                # stride 170: 3 chunks of <=170 columns fit one 512-col bank
                pS = psum_s.tile([128, len(bchunks), 170], FP32, tag="ps")
                s_prodb = junkp.tile([HB, 2, 256], FP32, tag="prod")
                for j, s in enumerate(range(s0, s1)):
                    w = L.W[s]
                    soff = int(L.OFF[s])
                    vb = VP1 * j
                    # prod = h1aug * v per column; d = colsum via ones-matmul
                    nc.vector.tensor_mul(
                        s_prodb[:, j, 0:w],
                        s_h1T.bitcast(FP32)[0:HB, soff : soff + w],
                        pv[vb : vb + HB, soff - boff : soff - boff + w],
                    )
                    for (cs, coff, cw, ci) in bchunks:
                        if cs != s:
                            continue
                        nc.tensor.matmul(
                            pd[0:cw, ci : ci + 1],
                            lhsT=s_prodb[:, j, coff : coff + cw],
                            rhs=s_ones[0:HB, :],
                            start=True,
                            stop=True,
                        )
                        ck = ci - bchunks[0][3]
                        nc.tensor.matmul(
                            pS[0:cw, ck, 0:w],
                            lhsT=s_h1T[:, soff + coff : soff + coff + cw],
                            rhs=s_v16[:, soff : soff + w],
                            start=True,
                            stop=True,
                        )
                        # sum_j relu(S) straight from PSUM (accum_out's
                        # reduction op is op1 -> must stay add); the 1/n mean
                        # folds into the host-side final
                        jk = junkp.tile([128, 256], FP32, tag="junk")
                        acc = s_out[0:cw, 0, ci : ci + 1]
                        if ck == 1:
                            nc.scalar.activation(
                                out=jk[0:cw, 0:w], in_=pS[0:cw, ck, 0:w],
                                func=F.Relu, accum_out=acc,
                            )
                        else:
                            nc.vector.tensor_scalar(
                                out=jk[0:cw, 0:w], in0=pS[0:cw, ck, 0:w],
                                scalar1=0.0, scalar2=None, op0=OP.max, op1=OP.add,
                                accum_out=acc,
                            )

            # d column straight out of PSUM into the output tile
            nc.vector.tensor_copy(s_out[:, 1, :], pd[:])
            nc.sync.dma_start(
                out=d_yout.rearrange("(p t c) -> p t c", p=128, t=2), in_=s_out[:]
            )

    nc.compile()
    return nc


def get_program(L: Layout):
    k = L.key()
    if k not in _PROGRAMS:
        _PROGRAMS[k] = _build_program(L)
    return _PROGRAMS[k]


# ---------------------------------------------------------------- host side
def _assign(cf):
    """Rank-sort categories; rank group g goes to slot position POS[g] so
    adjacent slot pairs (the matmul blocks) are >= 256 wide."""
    sizes = np.array([(cf == k).sum() for k in range(C)])
    order = np.argsort(-sizes, kind="stable")
    pos_of_group = [0, 2, 4, 6, 7, 5, 3, 1]
    widths = [0] * G
    catmap = [[0] * G for _ in range(NCORES)]
    nmap = [[0] * G for _ in range(NCORES)]
    for g in range(G):
        grp = order[8 * g : 8 * g + 8]
        p = pos_of_group[g]
        widths[p] = int(sizes[grp[0]])
        for core in range(NCORES):
            catmap[core][p] = int(grp[core])
            nmap[core][p] = int(sizes[grp[core]])
    return widths, catmap, nmap


def _prep_core_inputs(L, x, z, Ws, W1, b1, W2, b2, Wz, bz, idx_lists, catmap_c, nmap_c):
    xz = np.zeros((KZ * 128, L.R), np.float16)
    mrow = np.zeros(L.R, np.float32)
    for s in range(G):
        idx = idx_lists[catmap_c[s]]
        n = nmap_c[s]
        lo = int(L.OFF[s])
        if n:
            zT = z[idx].T
            zh = zT.astype(np.float16)
            xz[0:128, lo : lo + n] = zh
            xz[128:256, lo : lo + n] = (zT - zh.astype(np.float32)).astype(np.float16)
            xz[256:, lo : lo + n] = x[idx].T.astype(np.float16)
            mrow[lo : lo + n] = 1.0

    NB = len(L.blocks)
    packA = np.zeros((128, L.PW), np.float16)
    packA[:, L.PK_W1[0] : L.PK_W1[1]] = (
        W1.reshape(KX, 128, H).transpose(1, 0, 2).reshape(128, KX * H)
    ).astype(np.float16)
    packA[:, L.PK_WZ[0] : L.PK_WZ[1]] = Wz.astype(np.float16)
    # v weights: per block, [Z, 115] = [W2s_aug(s0) | zeros | W2s_aug(s1)]
    # where W2s_aug[g] = [ (W2 Ws[g])^T cols ; (b2 Ws[g]) ] laid out [Z, 51]
    w2T = np.zeros((128, NB, VP1 + HB), np.float32)
    for bi, (s0, s1) in enumerate(L.blocks):
        for j, s in enumerate(range(s0, s1)):
            Wsg = Ws[catmap_c[s]].astype(np.float64)
            blk = np.zeros((128, HB))
            blk[:, :H] = (W2.astype(np.float64) @ Wsg).T
            blk[:, H] = b2.astype(np.float64) @ Wsg
            w2T[:, bi, VP1 * j : VP1 * j + HB] = blk
    packA[:, L.PK_W2T[0] : L.PK_W2T[1]] = w2T.reshape(128, -1).astype(np.float16)
    packA[0, L.PK_B1R[0] : L.PK_B1R[1]] = b1.astype(np.float16)
    packA[0, L.PK_BZ[0] : L.PK_BZ[1]] = bz.astype(np.float16)
    packA[0, L.PK_MROW[0] : L.PK_MROW[1]] = mrow.astype(np.float16)

    return {"xz": xz, "packA": packA, "hones": mrow.reshape(1, -1)}


def _unpack_core_output(L, y, idx_lists, catmap_c, nmap_c, out):
    """y flat [(p t c)] -> rows; final log(softplus(d)+eps)-log(mean+eps) in
    float64 on the host (O(N) unshard-time scalar work)."""
    y = np.asarray(y).reshape(128, 2, L.NCHUNK).astype(np.float64)
    rel = y[:, 0, :]
    d = y[:, 1, :]
    T = np.log1p(np.exp(-np.abs(d))) + np.maximum(d, 0.0)
    logT = np.log(T + EPS)
    for (s, coff, cw, ci) in L.chunks:
        n = nmap_c[s]
        take = min(cw, n - coff)
        if take > 0:
            idx = idx_lists[catmap_c[s]][coff : coff + take]
            out[idx] = logT[0:take, ci] - np.log(rel[0:take, ci] / n + EPS)


def _numpy_fallback(x, c, z, W1, b1, W2, b2, Wz, bz, Ws):
    x64 = x.astype(np.float64)
    fx = np.maximum(x64 @ W1.astype(np.float64) + b1, 0.0) @ W2.astype(
        np.float64
    ) + b2
    fz = z.astype(np.float64) @ Wz.astype(np.float64) + bz
    u = np.einsum("nd,nde->ne", fx, Ws.astype(np.float64)[c])

    def sp(v):
        return np.log1p(np.exp(-np.abs(v))) + np.maximum(v, 0.0)

    T = sp(np.einsum("ne,ne->n", u, fz))
    out = np.empty(N, np.float64)
    for k in range(C):
        idx = np.where(c == k)[0]
        if len(idx) == 0:
            continue
        Sk = sp(u[idx] @ fz[idx].T)
        out[idx] = np.log(T[idx] + EPS) - np.log(Sk.mean(axis=1) + EPS)
    return out.astype(np.float32)


def kernel(x, c, z, W1, b1, W2, b2, Wz, bz, Ws):
    x = np.ascontiguousarray(np.asarray(x), dtype=np.float32)
    z = np.ascontiguousarray(np.asarray(z), dtype=np.float32)
    W1 = np.ascontiguousarray(np.asarray(W1), dtype=np.float32)
    b1 = np.ascontiguousarray(np.asarray(b1), dtype=np.float32)
    W2 = np.ascontiguousarray(np.asarray(W2), dtype=np.float32)
    b2 = np.ascontiguousarray(np.asarray(b2), dtype=np.float32)
    Wz = np.ascontiguousarray(np.asarray(Wz), dtype=np.float32)
    bz = np.ascontiguousarray(np.asarray(bz), dtype=np.float32)
    Ws = np.ascontiguousarray(np.asarray(Ws), dtype=np.float32)
    cf = np.asarray(c).reshape(-1).astype(np.int64)

    idx_lists = [np.where(cf == k)[0] for k in range(C)]
    sizes = [len(i) for i in idx_lists]
    if max(sizes) > 256 or min(sizes) == 0 or len(cf) != N:
        return _numpy_fallback(x, cf, z, W1, b1, W2, b2, Wz, bz, Ws)

    widths, catmap, nmap = _assign(cf)
    L = Layout(widths)
    if not L.ok():
        return _numpy_fallback(x, cf, z, W1, b1, W2, b2, Wz, bz, Ws)

    in_maps = [
        _prep_core_inputs(
            L, x, z, Ws, W1, b1, W2, b2, Wz, bz, idx_lists, catmap[core], nmap[core]
        )
        for core in range(NCORES)
    ]

    nc = get_program(L)
    res = run_bass_kernel_spmd(nc, in_maps, core_ids=list(range(NCORES)))

    out = np.empty(N, np.float32)
    for core in range(NCORES):
        _unpack_core_output(
            L, res.results[core]["yout"], idx_lists, catmap[core], nmap[core], out
        )
    return out


# revision 23
# speedup vs baseline: 1.2741x; 1.0820x over previous
"""Trainium2 Bass kernel for the CPC contrastive loss problem.

Math (reference):
    fx = relu(x @ W1 + b1) @ W2 + b2          [N, Z]
    fz = z @ Wz + bz                          [N, Z]
    u[n] = fx[n] @ Ws[c[n]]                   [N, Z]
    T = softplus(<u, fz>_row)                 [N]
    neg_T[i] = mean_{j: c[j]==c[i]} softplus(<u[i], fz[j]>)
    out = log(T + eps) - log(neg_T + eps)

Structure: rows are grouped by category on the host; each of the 8 cores gets
8 categories, so the NxN S matrix reduces to per-category blocks (64x less
work). Categories are rank-sorted by size; slot s holds same-rank categories
on every core, so the slot widths W[s] (max size in the rank group) bake into
one SPMD program. Slot positions interleave large/small ranks so adjacent
pairs (the processing blocks) are >= 256 columns wide: matmuls below 256
output columns can run at reduced rate.

Key algebra: with the augmented fold W2s_aug[g] = [W2 Ws[g]; b2 Ws[g]] and
h1aug = [relu(x W1 + b1); 1],
    S = h1aug^T v,   d_i = <h1aug_i, v_i>,   v_j = W2s_aug fz_j,
and since fz is consumed ONLY through v, Wz/bz fold in on the host:
    v = (W2s_aug Wz^T) z + (W2s_aug bz) x mrow.
So the device runs just two matmul stages per block (v from z, h1 from x)
plus the bf16-free S/d stage. No u stage, no fz stage at all.

Other optimizations:
  - x, z, and all folded weights ship/compute in fp16: same 10-bit mantissa
    as the fp32r (tf32-like) mode, so accuracy is unchanged (~7e-4 measured
    vs the 2e-2 budget) while DMA halves. z rides as an fp16 hi+lo pair
    (exact to fp32) because d needs full input precision. b1 and the v bias
    fold in via rank-1 matmuls against the valid-row mask, which also keeps
    padded columns exactly zero.
  - neg_T uses relu instead of softplus: S entries have std ~89, so the
    log1p(exp(-|S|)) correction inside a 100+-term mean inside a log is
    ~2e-5 relative. This deletes the whole Abs/Exp/Ln/reduce tail over S.
  - One DMA per block (z pair + x k-chunks stacked in one fp16 tensor);
    the cost model charges ~650ns issue + ~625ns HWDGE per DMA, so few
    large transfers win. The last block splits z early / x per k-chunk so
    its v chain clears and h1 accumulates while data streams in.
  - Both slots' v weights batch into one [128, 115] stationary operand
    (slot 1's rows at partition 64: partition starts must be 32-aligned;
    per-slot copies bring them back to base 0 for the S matmuls).
  - The device returns d and sum_j relu(S) per row; the final
    log(softplus(d)+eps) - log(mean+eps) is O(N) float64 work on the host
    during unsharding (exact softplus, no LUT range issues).
  - S matmuls are emitted before the d matmuls so the in-order PE queue
    never waits on the DVE h1*v products mid-stage.
"""

import sys

for _p in ("/opt/trn_rl_repo", "/root/.axon_site/_ro/trn_rl_repo"):
    if _p not in sys.path:
        sys.path.append(_p)

import numpy as np

import concourse.bacc as bacc
import concourse.tile as tile
from concourse import mybir as mb
from concourse.bass_utils import run_bass_kernel_spmd

# ---------------------------------------------------------------- constants
N, IN, Z, C, H = 8192, 512, 128, 64, 50
NCORES = 8
G = C // NCORES          # category slots per core
KX = IN // 128           # k-tiles for x
KZ = 2 + KX              # fp16 row-groups in the xz tensor: zh zl x0..x3
EPS = 1e-8
N_WARM = 8
HB = H + 1               # 51: h1 rows plus the ones row at partition H
VP1 = 64                 # partition base of slot-1's v rows (multiple of 32)

F = mb.ActivationFunctionType
OP = mb.AluOpType
FP32 = mb.dt.float32
FP32R = mb.dt.float32r
FP16 = mb.dt.float16
BF16 = mb.dt.bfloat16

_PROGRAMS = {}


class Layout:
    """Slot/chunk/block geometry baked into the program (shared by cores)."""

    def __init__(self, widths):
        assert len(widths) == G
        self.W = list(widths)
        self.OFF = np.concatenate([[0], np.cumsum(self.W)]).astype(int)
        self.R = int(self.OFF[-1])
        # chunks: (slot, coff, cw, ci)
        self.chunks = []
        for s, w in enumerate(self.W):
            for coff in range(0, w, 128):
                self.chunks.append((s, coff, min(128, w - coff), len(self.chunks)))
        self.NCHUNK = len(self.chunks)
        self.blocks = [(s, min(s + 2, G)) for s in range(0, G, 2)]
        NB = len(self.blocks)
        # packA column layout (fp16; W1/M span partitions, the rest are row-0)
        o = 0
        self.PK_W1 = (o, o + KX * H)
        o += KX * H
        self.PK_M = (o, o + NB * (VP1 + HB))
        o += NB * (VP1 + HB)
        self.PK_B1R = (o, o + H)
        o += H
        self.PK_CB = (o, o + NB * (VP1 + HB))
        o += NB * (VP1 + HB)
        self.PK_MROW = (o, o + self.R)
        self.PW = o + self.R

    def ok(self):
        return all(
            int(self.OFF[s1] - self.OFF[s0]) >= 256 for s0, s1 in self.blocks
        ) and max(self.W) <= 170 and all(s1 - s0 == 2 for s0, s1 in self.blocks)

    def key(self):
        return tuple(self.W)


def _build_program(L: Layout):
    nc = bacc.Bacc("TRN2", target_bir_lowering=False, debug=False)

    R, NC_ = L.R, L.NCHUNK
    d_xz = nc.dram_tensor("xz", [KZ * 128, R], FP16, kind="ExternalInput").ap()
    d_packA = nc.dram_tensor("packA", [128, L.PW], FP16, kind="ExternalInput").ap()
    d_hones = nc.dram_tensor("hones", [1, R], FP32, kind="ExternalInput").ap()
    d_yout = nc.dram_tensor("yout", [128 * 2 * NC_], FP32, kind="ExternalOutput").ap()

    xz_view = d_xz.rearrange("(k p) n -> p k n", p=128)
    NB = len(L.blocks)

    with tile.TileContext(nc) as tc:
        with (
            tc.tile_pool(name="const", bufs=1) as const,
            tc.tile_pool(name="junk", bufs=3) as junkp,
            tc.tile_pool(name="psum_h", bufs=2, space="PSUM") as psum_h,
            tc.tile_pool(name="psum_v", bufs=2, space="PSUM") as psum_v,
            tc.tile_pool(name="psum_s", bufs=2, space="PSUM") as psum_s,
            tc.tile_pool(name="psum_d", bufs=1, space="PSUM") as psum_d,
        ):
            # ---- constants
            s_ones = const.tile([128, 1], FP32)
            nc.vector.memset(s_ones[:], 1.0)
            # the one ACT table set (id 6) holding Copy/Relu used below
            nc.scalar.add_instruction(
                mb.InstLoadActFuncSet(
                    name=nc.get_next_instruction_name(),
                    ins=[],
                    outs=[],
                    act_func_set_id=6,
                )
            )
            s_warmact = const.tile([128, 1], FP32)
            nc.scalar.activation(out=s_warmact[:], in_=s_ones[:], func=F.Abs)

            # ---- persistent tiles
            s_xz = const.tile([128, KZ, R], FP16)
            s_h1T = const.tile([HB, R], FP32R)
            s_v16 = const.tile([HB, R], FP16)
            s_out = const.tile([128, 2, NC_], FP32)  # [:,0,:] relu-sums, [:,1,:] d
            s_packA = const.tile([128, L.PW], FP16)

            # ---- all DMAs up front in issue order
            nc.sync.dma_start(out=s_packA[:], in_=d_packA[:])
            nc.sync.dma_start(out=s_h1T.bitcast(FP32)[H : H + 1, :], in_=d_hones[:])
            for bi, (s0, s1) in enumerate(L.blocks):
                ns = slice(int(L.OFF[s0]), int(L.OFF[s1]))
                if bi == NB - 1:
                    # z first, then x per k-chunk: the v chain clears early
                    # and h1 accumulates while x streams in
                    nc.sync.dma_start(out=s_xz[:, 0:2, ns], in_=xz_view[:, 0:2, ns])
                    for k in range(KX):
                        nc.sync.dma_start(
                            out=s_xz[:, 2 + k, ns], in_=xz_view[:, 2 + k, ns]
                        )
                else:
                    nc.sync.dma_start(out=s_xz[:, :, ns], in_=xz_view[:, :, ns])

            s_w1 = s_packA[:, L.PK_W1[0] : L.PK_W1[1]].rearrange(
                "p (k h) -> p k h", k=KX
            )
            s_M = s_packA[:, L.PK_M[0] : L.PK_M[1]].rearrange("p (b q) -> p b q", b=NB)
            s_b1r = s_packA[0:1, L.PK_B1R[0] : L.PK_B1R[1]]
            s_cb = s_packA[0:1, L.PK_CB[0] : L.PK_CB[1]].rearrange(
                "p (b q) -> p b q", b=NB
            )
            s_mrow = s_packA[0:1, L.PK_MROW[0] : L.PK_MROW[1]]

            # PE warm-up to start the p-state ramp while DMA runs
            pwarm = psum_v.tile([1, 64], FP32, tag="pv")
            s_wrhs = const.tile([128, 64], FP32)
            nc.vector.memset(s_wrhs[:], 0.0)
            for _ in range(N_WARM):
                nc.tensor.matmul(
                    pwarm[:], lhsT=s_ones[:], rhs=s_wrhs[:], start=True, stop=True
                )

            # chunks narrower than 128 leave tail partitions untouched
            nc.vector.memset(s_out[:], 0.0)
            pd = psum_d.tile([128, NC_], FP32)
            nc.vector.memset(pd[:], 0.0)

            for bi, (s0, s1) in enumerate(L.blocks):
                boff = int(L.OFF[s0])
                bw = int(L.OFF[s1] - L.OFF[s0])
                ns = slice(boff, boff + bw)

                # v = M z + c x mrow for both slots at once: rows 0:51 slot
                # s0, rows 64:115 slot s1 (32-aligned partition bases). The
                # rank-1 bias leads so the z matmuls close the group.
                pv = psum_v.tile([VP1 + HB, bw], FP32, tag="pv")
                nc.tensor.matmul(
                    pv[:], lhsT=s_cb[:, bi, :], rhs=s_mrow[:, ns],
                    start=True, stop=False,
                )
                nc.tensor.matmul(
                    pv[:], lhsT=s_M[:, bi, :], rhs=s_xz[:, 0, ns],
                    start=False, stop=False,
                )
                nc.tensor.matmul(
                    pv[:], lhsT=s_M[:, bi, :], rhs=s_xz[:, 1, ns],
                    start=False, stop=True,
                )
                # per-slot copies shift slot 1's rows down to partition base 0
                # (matmul needs lhsT/rhs bases to match)
                for j, s in enumerate(range(s0, s1)):
                    w = L.W[s]
                    so = int(L.OFF[s]) - boff
                    nc.gpsimd.tensor_copy(
                        s_v16[:, boff + so : boff + so + w],
                        pv[VP1 * j : VP1 * j + HB, so : so + w],
                    )

                # h1 = relu(W1^T x + b1 x mrow); bias mm first so the last
                # x k-chunk is the only gate on closing the group
                ph = psum_h.tile([H, bw], FP32, tag="ph")
                nc.tensor.matmul(
                    ph[:], lhsT=s_b1r, rhs=s_mrow[:, ns], start=True, stop=False
                )
                for k in range(KX):
                    nc.tensor.matmul(
                        ph[:],
                        lhsT=s_w1[:, k, :],
                        rhs=s_xz[:, 2 + k, ns],
                        start=False,
                        stop=(k == KX - 1),
                    )
                nc.scalar.activation(
                    out=s_h1T.bitcast(FP32)[0:H, ns], in_=ph[:], func=F.Relu
                )

                # d products on DVE while the S matmuls run
                s_prodb = junkp.tile([HB, 2, 256], FP32, tag="prod")
                for j, s in enumerate(range(s0, s1)):
                    w = L.W[s]
                    soff = int(L.OFF[s])
                    nc.vector.tensor_mul(
                        s_prodb[:, j, 0:w],
                        s_h1T.bitcast(FP32)[0:HB, soff : soff + w],
                        pv[VP1 * j : VP1 * j + HB, soff - boff : soff - boff + w],
                    )

                # S chunks (stride 170: three <=170-col chunks in one bank),
                # then relu row-sums; d matmuls last (they wait on DVE)
                bchunks = [ch for ch in L.chunks if s0 <= ch[0] < s1]
                pS = psum_s.tile([128, len(bchunks), 170], FP32, tag="ps")
                for (cs, coff, cw, ci) in bchunks:
                    soff = int(L.OFF[cs])
                    w = L.W[cs]
                    ck = ci - bchunks[0][3]
                    nc.tensor.matmul(
                        pS[0:cw, ck, 0:w],
                        lhsT=s_h1T[:, soff + coff : soff + coff + cw],
                        rhs=s_v16[:, soff : soff + w],
                        start=True,
                        stop=True,
                    )
                    # sum_j relu(S) straight from PSUM (accum_out's reduction
                    # op is op1 -> must stay add); the 1/n mean folds into the
                    # host-side final
                    jk = junkp.tile([128, 256], FP32, tag="junk")
                    acc = s_out[0:cw, 0, ci : ci + 1]
                    if ck == 1:
                        nc.scalar.activation(
                            out=jk[0:cw, 0:w], in_=pS[0:cw, ck, 0:w],
                            func=F.Relu, accum_out=acc,
                        )
                    else:
                        nc.vector.tensor_scalar(
                            out=jk[0:cw, 0:w], in0=pS[0:cw, ck, 0:w],
                            scalar1=0.0, scalar2=None, op0=OP.max, op1=OP.add,
                            accum_out=acc,
                        )
                for (cs, coff, cw, ci) in bchunks:
                    j = cs - s0
                    nc.tensor.matmul(
                        pd[0:cw, ci : ci + 1],
                        lhsT=s_prodb[:, j, coff : coff + cw],
                        rhs=s_ones[0:HB, :],
                        start=True,
                        stop=True,
                    )

            # d column straight out of PSUM into the output tile
            nc.vector.tensor_copy(s_out[:, 1, :], pd[:])
            nc.sync.dma_start(
                out=d_yout.rearrange("(p t c) -> p t c", p=128, t=2), in_=s_out[:]
            )

    nc.compile()
    return nc


def get_program(L: Layout):
    k = L.key()
    if k not in _PROGRAMS:
        _PROGRAMS[k] = _build_program(L)
    return _PROGRAMS[k]


# ---------------------------------------------------------------- host side
def _assign(cf):
    """Rank-sort categories; rank group g goes to slot position POS[g] so
    adjacent slot pairs (the matmul blocks) are >= 256 wide."""
    sizes = np.array([(cf == k).sum() for k in range(C)])
    order = np.argsort(-sizes, kind="stable")
    pos_of_group = [0, 2, 4, 6, 7, 5, 3, 1]
    widths = [0] * G
    catmap = [[0] * G for _ in range(NCORES)]
    nmap = [[0] * G for _ in range(NCORES)]
    for g in range(G):
        grp = order[8 * g : 8 * g + 8]
        p = pos_of_group[g]
        widths[p] = int(sizes[grp[0]])
        for core in range(NCORES):
            catmap[core][p] = int(grp[core])
            nmap[core][p] = int(sizes[grp[core]])
    return widths, catmap, nmap


def _prep_core_inputs(L, x, z, Ws, W1, b1, W2, b2, Wz, bz, idx_lists, catmap_c, nmap_c):
    xz = np.zeros((KZ * 128, L.R), np.float16)
    mrow = np.zeros(L.R, np.float32)
    for s in range(G):
        idx = idx_lists[catmap_c[s]]
        n = nmap_c[s]
        lo = int(L.OFF[s])
        if n:
            zT = z[idx].T
            zh = zT.astype(np.float16)
            xz[0:128, lo : lo + n] = zh
            xz[128:256, lo : lo + n] = (zT - zh.astype(np.float32)).astype(np.float16)
            xz[256:, lo : lo + n] = x[idx].T.astype(np.float16)
            mrow[lo : lo + n] = 1.0

    NB = len(L.blocks)
    packA = np.zeros((128, L.PW), np.float16)
    packA[:, L.PK_W1[0] : L.PK_W1[1]] = (
        W1.reshape(KX, 128, H).transpose(1, 0, 2).reshape(128, KX * H)
    ).astype(np.float16)
    # v weights: per block [Z, 115] = [M(s0)^T | zeros | M(s1)^T] where
    # M[g] = W2s_aug[g] Wz^T, c[g] = W2s_aug[g] bz, W2s_aug = [W2 Ws; b2 Ws]
    Wz64 = Wz.astype(np.float64)
    Mpk = np.zeros((128, NB, VP1 + HB))
    cpk = np.zeros((1, NB, VP1 + HB))
    for bi, (s0, s1) in enumerate(L.blocks):
        for j, s in enumerate(range(s0, s1)):
            Wsg = Ws[catmap_c[s]].astype(np.float64)
            aug = np.zeros((HB, Z))
            aug[:H] = W2.astype(np.float64) @ Wsg
            aug[H] = b2.astype(np.float64) @ Wsg
            Mpk[:, bi, VP1 * j : VP1 * j + HB] = (aug @ Wz64.T).T
            cpk[0, bi, VP1 * j : VP1 * j + HB] = aug @ bz.astype(np.float64)
    packA[:, L.PK_M[0] : L.PK_M[1]] = Mpk.reshape(128, -1).astype(np.float16)
    packA[0, L.PK_B1R[0] : L.PK_B1R[1]] = b1.astype(np.float16)
    packA[0, L.PK_CB[0] : L.PK_CB[1]] = cpk.reshape(-1).astype(np.float16)
    packA[0, L.PK_MROW[0] : L.PK_MROW[1]] = mrow.astype(np.float16)

    return {"xz": xz, "packA": packA, "hones": mrow.reshape(1, -1)}


def _unpack_core_output(L, y, idx_lists, catmap_c, nmap_c, out):
    """y flat [(p t c)] -> rows; final log(softplus(d)+eps)-log(mean+eps) in
    float64 on the host (O(N) unshard-time scalar work)."""
    y = np.asarray(y).reshape(128, 2, L.NCHUNK).astype(np.float64)
    rel = y[:, 0, :]
    d = y[:, 1, :]
    T = np.log1p(np.exp(-np.abs(d))) + np.maximum(d, 0.0)
    logT = np.log(T + EPS)
    for (s, coff, cw, ci) in L.chunks:
        n = nmap_c[s]
        take = min(cw, n - coff)
        if take > 0:
            idx = idx_lists[catmap_c[s]][coff : coff + take]
            out[idx] = logT[0:take, ci] - np.log(rel[0:take, ci] / n + EPS)


def _numpy_fallback(x, c, z, W1, b1, W2, b2, Wz, bz, Ws):
    x64 = x.astype(np.float64)
    fx = np.maximum(x64 @ W1.astype(np.float64) + b1, 0.0) @ W2.astype(
        np.float64
    ) + b2
    fz = z.astype(np.float64) @ Wz.astype(np.float64) + bz
    u = np.einsum("nd,nde->ne", fx, Ws.astype(np.float64)[c])

    def sp(v):
        return np.log1p(np.exp(-np.abs(v))) + np.maximum(v, 0.0)

    T = sp(np.einsum("ne,ne->n", u, fz))
    out = np.empty(N, np.float64)
    for k in range(C):
        idx = np.where(c == k)[0]
        if len(idx) == 0:
            continue
        Sk = sp(u[idx] @ fz[idx].T)
        out[idx] = np.log(T[idx] + EPS) - np.log(Sk.mean(axis=1) + EPS)
    return out.astype(np.float32)


def kernel(x, c, z, W1, b1, W2, b2, Wz, bz, Ws):
    x = np.ascontiguousarray(np.asarray(x), dtype=np.float32)
    z = np.ascontiguousarray(np.asarray(z), dtype=np.float32)
    W1 = np.ascontiguousarray(np.asarray(W1), dtype=np.float32)
    b1 = np.ascontiguousarray(np.asarray(b1), dtype=np.float32)
    W2 = np.ascontiguousarray(np.asarray(W2), dtype=np.float32)
    b2 = np.ascontiguousarray(np.asarray(b2), dtype=np.float32)
    Wz = np.ascontiguousarray(np.asarray(Wz), dtype=np.float32)
    bz = np.ascontiguousarray(np.asarray(bz), dtype=np.float32)
    Ws = np.ascontiguousarray(np.asarray(Ws), dtype=np.float32)
    cf = np.asarray(c).reshape(-1).astype(np.int64)

    idx_lists = [np.where(cf == k)[0] for k in range(C)]
    sizes = [len(i) for i in idx_lists]
    if max(sizes) > 256 or min(sizes) == 0 or len(cf) != N:
        return _numpy_fallback(x, cf, z, W1, b1, W2, b2, Wz, bz, Ws)

    widths, catmap, nmap = _assign(cf)
    L = Layout(widths)
    if not L.ok():
        return _numpy_fallback(x, cf, z, W1, b1, W2, b2, Wz, bz, Ws)

    in_maps = [
        _prep_core_inputs(
            L, x, z, Ws, W1, b1, W2, b2, Wz, bz, idx_lists, catmap[core], nmap[core]
        )
        for core in range(NCORES)
    ]

    nc = get_program(L)
    res = run_bass_kernel_spmd(nc, in_maps, core_ids=list(range(NCORES)))

    out = np.empty(N, np.float32)
    for core in range(NCORES):
        _unpack_core_output(
            L, res.results[core]["yout"], idx_lists, catmap[core], nmap[core], out
        )
    return out


# revision 25
# speedup vs baseline: 1.3740x; 1.0784x over previous
"""Trainium2 Bass kernel for the CPC contrastive loss problem.

Math (reference):
    fx = relu(x @ W1 + b1) @ W2 + b2          [N, Z]
    fz = z @ Wz + bz                          [N, Z]
    u[n] = fx[n] @ Ws[c[n]]                   [N, Z]
    T = softplus(<u, fz>_row)                 [N]
    neg_T[i] = mean_{j: c[j]==c[i]} softplus(<u[i], fz[j]>)
    out = log(T + eps) - log(neg_T + eps)

Structure: rows are grouped by category on the host; each of the 8 cores gets
8 categories, so the NxN S matrix reduces to per-category blocks (64x less
work). Categories are rank-sorted by size; slot s holds same-rank categories
on every core, so the slot widths W[s] (max size in the rank group) bake into
one SPMD program. Slot positions interleave large/small ranks so adjacent
pairs (the processing blocks) are >= 256 columns wide: matmuls below 256
output columns can run at reduced rate.

Key algebra: with the augmented fold W2s_aug[g] = [W2 Ws[g]; b2 Ws[g]] and
h1aug = [relu(x W1 + b1); 1],
    S = h1aug^T v,   d_i = <h1aug_i, v_i>,   v_j = W2s_aug fz_j,
and since fz is consumed ONLY through v, Wz/bz fold in on the host:
    v = (W2s_aug Wz^T) z + (W2s_aug bz) x mrow.
So the device runs just two matmul stages per block (v from z, h1 from x)
plus the bf16-free S/d stage. No u stage, no fz stage at all.

Other optimizations:
  - x, z, and all folded weights ship/compute in fp16: same 10-bit mantissa
    as the fp32r (tf32-like) mode, so accuracy is unchanged (~7e-4 measured
    vs the 2e-2 budget) while DMA halves. z rides as an fp16 hi+lo pair
    (exact to fp32) because d needs full input precision. b1 and the v bias
    fold in via rank-1 matmuls against the valid-row mask, which also keeps
    padded columns exactly zero.
  - neg_T uses relu instead of softplus: S entries have std ~89, so the
    log1p(exp(-|S|)) correction inside a 100+-term mean inside a log is
    ~2e-5 relative. This deletes the whole Abs/Exp/Ln/reduce tail over S.
  - One DMA per block (z pair + x k-chunks stacked in one fp16 tensor);
    the cost model charges ~650ns issue + ~625ns HWDGE per DMA, so few
    large transfers win. The last block splits z early / x per k-chunk so
    its v chain clears and h1 accumulates while data streams in.
  - Both slots' v weights batch into one [128, 115] stationary operand
    (slot 1's rows at partition 64: partition starts must be 32-aligned;
    per-slot copies bring them back to base 0 for the S matmuls).
  - The device returns d and sum_j relu(S) per row; the final
    log(softplus(d)+eps) - log(mean+eps) is O(N) float64 work on the host
    during unsharding (exact softplus, no LUT range issues).
  - S matmuls are emitted before the d matmuls so the in-order PE queue
    never waits on the DVE h1*v products mid-stage.
"""

import sys

for _p in ("/opt/trn_rl_repo", "/root/.axon_site/_ro/trn_rl_repo"):
    if _p not in sys.path:
        sys.path.append(_p)

import numpy as np

import concourse.bacc as bacc
import concourse.tile as tile
from concourse import mybir as mb
from concourse.bass_utils import run_bass_kernel_spmd

# ---------------------------------------------------------------- constants
N, IN, Z, C, H = 8192, 512, 128, 64, 50
NCORES = 8
G = C // NCORES          # category slots per core
KX = IN // 128           # k-tiles for x
KZ = 2 + KX              # fp16 row-groups in the xz tensor: zh zl x0..x3
EPS = 1e-8
N_WARM = 8
HB = H + 1               # 51: h1 rows plus the ones row at partition H
VP1 = 64                 # partition base of slot-1's v rows (multiple of 32)

F = mb.ActivationFunctionType
OP = mb.AluOpType
FP32 = mb.dt.float32
FP32R = mb.dt.float32r
FP16 = mb.dt.float16
BF16 = mb.dt.bfloat16

_PROGRAMS = {}


class Layout:
    """Slot/chunk/block geometry baked into the program (shared by cores)."""

    def __init__(self, widths):
        assert len(widths) == G
        self.W = list(widths)
        self.OFF = np.concatenate([[0], np.cumsum(self.W)]).astype(int)
        self.R = int(self.OFF[-1])
        # chunks: (slot, coff, cw, ci)
        self.chunks = []
        for s, w in enumerate(self.W):
            for coff in range(0, w, 128):
                self.chunks.append((s, coff, min(128, w - coff), len(self.chunks)))
        self.NCHUNK = len(self.chunks)
        self.blocks = [(s, min(s + 2, G)) for s in range(0, G, 2)]
        NB = len(self.blocks)
        # packA column layout (fp16; W1/M span partitions, the rest are row-0)
        o = 0
        self.PK_W1 = (o, o + KX * H)
        o += KX * H
        self.PK_M = (o, o + NB * (VP1 + HB))
        o += NB * (VP1 + HB)
        self.PK_B1R = (o, o + H)
        o += H
        self.PK_CB = (o, o + NB * (VP1 + HB))
        o += NB * (VP1 + HB)
        self.PK_MROW = (o, o + self.R)
        self.PW = o + self.R

    def ok(self):
        return all(
            int(self.OFF[s1] - self.OFF[s0]) >= 256 for s0, s1 in self.blocks
        ) and max(self.W) <= 170 and all(s1 - s0 == 2 for s0, s1 in self.blocks)

    def key(self):
        return tuple(self.W)


def _build_program(L: Layout):
    nc = bacc.Bacc("TRN2", target_bir_lowering=False, debug=False)

    R, NC_ = L.R, L.NCHUNK
    d_xz = nc.dram_tensor("xz", [KZ * 128, R], FP16, kind="ExternalInput").ap()
    d_packA = nc.dram_tensor("packA", [128, L.PW], FP16, kind="ExternalInput").ap()
    d_hones = nc.dram_tensor("hones", [1, R], FP32, kind="ExternalInput").ap()
    d_yout = nc.dram_tensor("yout", [128 * 2 * NC_], FP32, kind="ExternalOutput").ap()

    xz_view = d_xz.rearrange("(k p) n -> p k n", p=128)
    NB = len(L.blocks)

    with tile.TileContext(nc) as tc:
        with (
            tc.tile_pool(name="const", bufs=1) as const,
            tc.tile_pool(name="junk", bufs=3) as junkp,
            tc.tile_pool(name="psum_h", bufs=2, space="PSUM") as psum_h,
            tc.tile_pool(name="psum_v", bufs=2, space="PSUM") as psum_v,
            tc.tile_pool(name="psum_s", bufs=2, space="PSUM") as psum_s,
            tc.tile_pool(name="psum_d", bufs=1, space="PSUM") as psum_d,
        ):
            # ---- constants
            s_ones = const.tile([128, 1], FP32)
            nc.vector.memset(s_ones[:], 1.0)
            # the one ACT table set (id 6) holding Copy/Relu used below
            nc.scalar.add_instruction(
                mb.InstLoadActFuncSet(
                    name=nc.get_next_instruction_name(),
                    ins=[],
                    outs=[],
                    act_func_set_id=6,
                )
            )
            s_warmact = const.tile([128, 1], FP32)
            nc.scalar.activation(out=s_warmact[:], in_=s_ones[:], func=F.Abs)

            # ---- persistent tiles
            s_xz = const.tile([128, KZ, R], FP16)
            s_h1T = const.tile([HB, R], FP32R)
            s_v16 = const.tile([HB, R], FP16)
            s_out = const.tile([128, 2, NC_], FP32)  # [:,0,:] relu-sums, [:,1,:] d
            s_packA = const.tile([128, L.PW], FP16)

            # ---- all DMAs up front in issue order
            nc.sync.dma_start(out=s_packA[:], in_=d_packA[:])
            nc.sync.dma_start(out=s_h1T.bitcast(FP32)[H : H + 1, :], in_=d_hones[:])
            for bi, (s0, s1) in enumerate(L.blocks):
                ns = slice(int(L.OFF[s0]), int(L.OFF[s1]))
                if bi == NB - 1:
                    # z first, then x per k-chunk: the v chain clears early
                    # and h1 accumulates while x streams in
                    nc.sync.dma_start(out=s_xz[:, 0:2, ns], in_=xz_view[:, 0:2, ns])
                    for k in range(KX):
                        nc.sync.dma_start(
                            out=s_xz[:, 2 + k, ns], in_=xz_view[:, 2 + k, ns]
                        )
                else:
                    nc.sync.dma_start(out=s_xz[:, :, ns], in_=xz_view[:, :, ns])

            s_w1 = s_packA[:, L.PK_W1[0] : L.PK_W1[1]].rearrange(
                "p (k h) -> p k h", k=KX
            )
            s_M = s_packA[:, L.PK_M[0] : L.PK_M[1]].rearrange("p (b q) -> p b q", b=NB)
            s_b1r = s_packA[0:1, L.PK_B1R[0] : L.PK_B1R[1]]
            s_cb = s_packA[0:1, L.PK_CB[0] : L.PK_CB[1]].rearrange(
                "p (b q) -> p b q", b=NB
            )
            s_mrow = s_packA[0:1, L.PK_MROW[0] : L.PK_MROW[1]]

            # PE warm-up to start the p-state ramp while DMA runs
            pwarm = psum_v.tile([1, 64], FP32, tag="pv")
            s_wrhs = const.tile([128, 64], FP32)
            nc.vector.memset(s_wrhs[:], 0.0)
            for _ in range(N_WARM):
                nc.tensor.matmul(
                    pwarm[:], lhsT=s_ones[:], rhs=s_wrhs[:], start=True, stop=True
                )

            # chunks narrower than 128 leave tail partitions untouched
            nc.vector.memset(s_out[:], 0.0)
            pd = psum_d.tile([128, NC_], FP32)
            nc.vector.memset(pd[:], 0.0)

            for bi, (s0, s1) in enumerate(L.blocks):
                boff = int(L.OFF[s0])
                bw = int(L.OFF[s1] - L.OFF[s0])
                ns = slice(boff, boff + bw)

                # v = M z + c x mrow for both slots at once: rows 0:51 slot
                # s0, rows 64:115 slot s1 (32-aligned partition bases). The
                # rank-1 bias leads so the z matmuls close the group.
                pv = psum_v.tile([VP1 + HB, bw], FP32, tag="pv")
                nc.tensor.matmul(
                    pv[:], lhsT=s_cb[:, bi, :], rhs=s_mrow[:, ns],
                    start=True, stop=False,
                )
                nc.tensor.matmul(
                    pv[:], lhsT=s_M[:, bi, :], rhs=s_xz[:, 0, ns],
                    start=False, stop=False,
                )
                nc.tensor.matmul(
                    pv[:], lhsT=s_M[:, bi, :], rhs=s_xz[:, 1, ns],
                    start=False, stop=True,
                )
                # per-slot copies shift slot 1's rows down to partition base 0
                # (matmul needs lhsT/rhs bases to match); ACT, since gpsimd
                # cannot touch PSUM on real hardware
                for j, s in enumerate(range(s0, s1)):
                    w = L.W[s]
                    so = int(L.OFF[s]) - boff
                    nc.scalar.activation(
                        out=s_v16[:, boff + so : boff + so + w],
                        in_=pv[VP1 * j : VP1 * j + HB, so : so + w],
                        func=F.Copy,
                    )

                # h1 = relu(W1^T x + b1 x mrow); bias mm first so the last
                # x k-chunk is the only gate on closing the group
                ph = psum_h.tile([H, bw], FP32, tag="ph")
                nc.tensor.matmul(
                    ph[:], lhsT=s_b1r, rhs=s_mrow[:, ns], start=True, stop=False
                )
                for k in range(KX):
                    nc.tensor.matmul(
                        ph[:],
                        lhsT=s_w1[:, k, :],
                        rhs=s_xz[:, 2 + k, ns],
                        start=False,
                        stop=(k == KX - 1),
                    )
                nc.scalar.activation(
                    out=s_h1T.bitcast(FP32)[0:H, ns], in_=ph[:], func=F.Relu
                )

                # d products on DVE while the S matmuls run
                s_prodb = junkp.tile([HB, 2, 256], FP32, tag="prod")
                for j, s in enumerate(range(s0, s1)):
                    w = L.W[s]
                    soff = int(L.OFF[s])
                    nc.vector.tensor_mul(
                        s_prodb[:, j, 0:w],
                        s_h1T.bitcast(FP32)[0:HB, soff : soff + w],
                        pv[VP1 * j : VP1 * j + HB, soff - boff : soff - boff + w],
                    )

                # S chunks (stride 170: three <=170-col chunks in one bank),
                # then relu row-sums; d matmuls last (they wait on DVE)
                bchunks = [ch for ch in L.chunks if s0 <= ch[0] < s1]
                pS = psum_s.tile([128, len(bchunks), 170], FP32, tag="ps")
                for (cs, coff, cw, ci) in bchunks:
                    soff = int(L.OFF[cs])
                    w = L.W[cs]
                    ck = ci - bchunks[0][3]
                    nc.tensor.matmul(
                        pS[0:cw, ck, 0:w],
                        lhsT=s_h1T[:, soff + coff : soff + coff + cw],
                        rhs=s_v16[:, soff : soff + w],
                        start=True,
                        stop=True,
                    )
                    # sum_j relu(S) straight from PSUM (accum_out's reduction
                    # op is op1 -> must stay add); the 1/n mean folds into the
                    # host-side final
                    jk = junkp.tile([128, 256], FP32, tag="junk")
                    acc = s_out[0:cw, 0, ci : ci + 1]
                    if ck == 2:
                        nc.scalar.activation(
                            out=jk[0:cw, 0:w], in_=pS[0:cw, ck, 0:w],
                            func=F.Relu, accum_out=acc,
                        )
                    else:
                        nc.vector.tensor_scalar(
                            out=jk[0:cw, 0:w], in0=pS[0:cw, ck, 0:w],
                            scalar1=0.0, scalar2=None, op0=OP.max, op1=OP.add,
                            accum_out=acc,
                        )
                for (cs, coff, cw, ci) in bchunks:
                    j = cs - s0
                    nc.tensor.matmul(
                        pd[0:cw, ci : ci + 1],
                        lhsT=s_prodb[:, j, coff : coff + cw],
                        rhs=s_ones[0:HB, :],
                        start=True,
                        stop=True,
                    )

            # d column straight out of PSUM into the output tile
            nc.vector.tensor_copy(s_out[:, 1, :], pd[:])
            nc.sync.dma_start(
                out=d_yout.rearrange("(p t c) -> p t c", p=128, t=2), in_=s_out[:]
            )

    nc.compile()
    return nc


def get_program(L: Layout):
    k = L.key()
    if k not in _PROGRAMS:
        _PROGRAMS[k] = _build_program(L)
    return _PROGRAMS[k]


# ---------------------------------------------------------------- host side
def _assign(cf):
    """Rank-sort categories; rank group g goes to slot position POS[g] so
    adjacent slot pairs (the matmul blocks) are >= 256 wide."""
    sizes = np.array([(cf == k).sum() for k in range(C)])
    order = np.argsort(-sizes, kind="stable")
    pos_of_group = [0, 2, 4, 6, 7, 5, 3, 1]
    widths = [0] * G
    catmap = [[0] * G for _ in range(NCORES)]
    nmap = [[0] * G for _ in range(NCORES)]
    for g in range(G):
        grp = order[8 * g : 8 * g + 8]
        p = pos_of_group[g]
        widths[p] = int(sizes[grp[0]])
        for core in range(NCORES):
            catmap[core][p] = int(grp[core])
            nmap[core][p] = int(sizes[grp[core]])
    return widths, catmap, nmap


def _prep_core_inputs(L, x, z, Ws, W1, b1, W2, b2, Wz, bz, idx_lists, catmap_c, nmap_c):
    xz = np.zeros((KZ * 128, L.R), np.float16)
    mrow = np.zeros(L.R, np.float32)
    for s in range(G):
        idx = idx_lists[catmap_c[s]]
        n = nmap_c[s]
        lo = int(L.OFF[s])
        if n:
            zT = z[idx].T
            zh = zT.astype(np.float16)
            xz[0:128, lo : lo + n] = zh
            xz[128:256, lo : lo + n] = (zT - zh.astype(np.float32)).astype(np.float16)
            xz[256:, lo : lo + n] = x[idx].T.astype(np.float16)
            mrow[lo : lo + n] = 1.0

    NB = len(L.blocks)
    packA = np.zeros((128, L.PW), np.float16)
    packA[:, L.PK_W1[0] : L.PK_W1[1]] = (
        W1.reshape(KX, 128, H).transpose(1, 0, 2).reshape(128, KX * H)
    ).astype(np.float16)
    # v weights: per block [Z, 115] = [M(s0)^T | zeros | M(s1)^T] where
    # M[g] = W2s_aug[g] Wz^T, c[g] = W2s_aug[g] bz, W2s_aug = [W2 Ws; b2 Ws]
    Wz64 = Wz.astype(np.float64)
    Mpk = np.zeros((128, NB, VP1 + HB))
    cpk = np.zeros((1, NB, VP1 + HB))
    for bi, (s0, s1) in enumerate(L.blocks):
        for j, s in enumerate(range(s0, s1)):
            Wsg = Ws[catmap_c[s]].astype(np.float64)
            aug = np.zeros((HB, Z))
            aug[:H] = W2.astype(np.float64) @ Wsg
            aug[H] = b2.astype(np.float64) @ Wsg
            Mpk[:, bi, VP1 * j : VP1 * j + HB] = (aug @ Wz64.T).T
            cpk[0, bi, VP1 * j : VP1 * j + HB] = aug @ bz.astype(np.float64)
    packA[:, L.PK_M[0] : L.PK_M[1]] = Mpk.reshape(128, -1).astype(np.float16)
    packA[0, L.PK_B1R[0] : L.PK_B1R[1]] = b1.astype(np.float16)
    packA[0, L.PK_CB[0] : L.PK_CB[1]] = cpk.reshape(-1).astype(np.float16)
    packA[0, L.PK_MROW[0] : L.PK_MROW[1]] = mrow.astype(np.float16)

    return {"xz": xz, "packA": packA, "hones": mrow.reshape(1, -1)}


def _unpack_core_output(L, y, idx_lists, catmap_c, nmap_c, out):
    """y flat [(p t c)] -> rows; final log(softplus(d)+eps)-log(mean+eps) in
    float64 on the host (O(N) unshard-time scalar work)."""
    y = np.asarray(y).reshape(128, 2, L.NCHUNK).astype(np.float64)
    rel = y[:, 0, :]
    d = y[:, 1, :]
    T = np.log1p(np.exp(-np.abs(d))) + np.maximum(d, 0.0)
    logT = np.log(T + EPS)
    for (s, coff, cw, ci) in L.chunks:
        n = nmap_c[s]
        take = min(cw, n - coff)
        if take > 0:
            idx = idx_lists[catmap_c[s]][coff : coff + take]
            out[idx] = logT[0:take, ci] - np.log(rel[0:take, ci] / n + EPS)


def _numpy_fallback(x, c, z, W1, b1, W2, b2, Wz, bz, Ws):
    x64 = x.astype(np.float64)
    fx = np.maximum(x64 @ W1.astype(np.float64) + b1, 0.0) @ W2.astype(
        np.float64
    ) + b2
    fz = z.astype(np.float64) @ Wz.astype(np.float64) + bz
    u = np.einsum("nd,nde->ne", fx, Ws.astype(np.float64)[c])

    def sp(v):
        return np.log1p(np.exp(-np.abs(v))) + np.maximum(v, 0.0)

    T = sp(np.einsum("ne,ne->n", u, fz))
    out = np.empty(N, np.float64)
    for k in range(C):
        idx = np.where(c == k)[0]
        if len(idx) == 0:
            continue
        Sk = sp(u[idx] @ fz[idx].T)
        out[idx] = np.log(T[idx] + EPS) - np.log(Sk.mean(axis=1) + EPS)
    return out.astype(np.float32)


def kernel(x, c, z, W1, b1, W2, b2, Wz, bz, Ws):
    x = np.ascontiguousarray(np.asarray(x), dtype=np.float32)
    z = np.ascontiguousarray(np.asarray(z), dtype=np.float32)
    W1 = np.ascontiguousarray(np.asarray(W1), dtype=np.float32)
    b1 = np.ascontiguousarray(np.asarray(b1), dtype=np.float32)
    W2 = np.ascontiguousarray(np.asarray(W2), dtype=np.float32)
    b2 = np.ascontiguousarray(np.asarray(b2), dtype=np.float32)
    Wz = np.ascontiguousarray(np.asarray(Wz), dtype=np.float32)
    bz = np.ascontiguousarray(np.asarray(bz), dtype=np.float32)
    Ws = np.ascontiguousarray(np.asarray(Ws), dtype=np.float32)
    cf = np.asarray(c).reshape(-1).astype(np.int64)

    idx_lists = [np.where(cf == k)[0] for k in range(C)]
    sizes = [len(i) for i in idx_lists]
    if max(sizes) > 256 or min(sizes) == 0 or len(cf) != N:
        return _numpy_fallback(x, cf, z, W1, b1, W2, b2, Wz, bz, Ws)

    widths, catmap, nmap = _assign(cf)
    L = Layout(widths)
    if not L.ok():
        return _numpy_fallback(x, cf, z, W1, b1, W2, b2, Wz, bz, Ws)

    in_maps = [
        _prep_core_inputs(
            L, x, z, Ws, W1, b1, W2, b2, Wz, bz, idx_lists, catmap[core], nmap[core]
        )
        for core in range(NCORES)
    ]

    nc = get_program(L)
    res = run_bass_kernel_spmd(nc, in_maps, core_ids=list(range(NCORES)))

    out = np.empty(N, np.float32)
    for core in range(NCORES):
        _unpack_core_output(
            L, res.results[core]["yout"], idx_lists, catmap[core], nmap[core], out
        )
    return out


# revision 26
# speedup vs baseline: 1.3745x; 1.0003x over previous
"""Trainium2 Bass kernel for the CPC contrastive loss problem.

Math (reference):
    fx = relu(x @ W1 + b1) @ W2 + b2          [N, Z]
    fz = z @ Wz + bz                          [N, Z]
    u[n] = fx[n] @ Ws[c[n]]                   [N, Z]
    T = softplus(<u, fz>_row)                 [N]
    neg_T[i] = mean_{j: c[j]==c[i]} softplus(<u[i], fz[j]>)
    out = log(T + eps) - log(neg_T + eps)

Structure: rows are grouped by category on the host; each of the 8 cores gets
8 categories, so the NxN S matrix reduces to per-category blocks (64x less
work). Categories are rank-sorted by size; slot s holds same-rank categories
on every core, so the slot widths W[s] (max size in the rank group) bake into
one SPMD program. Slot positions interleave large/small ranks so adjacent
pairs (the processing blocks) are >= 256 columns wide: matmuls below 256
output columns can run at reduced rate.

Key algebra: with the augmented fold W2s_aug[g] = [W2 Ws[g]; b2 Ws[g]] and
h1aug = [relu(x W1 + b1); 1],
    S = h1aug^T v,   d_i = <h1aug_i, v_i>,   v_j = W2s_aug fz_j,
and since fz is consumed ONLY through v, Wz/bz fold in on the host:
    v = (W2s_aug Wz^T) z + (W2s_aug bz) x mrow.
So the device runs just two matmul stages per block (v from z, h1 from x)
plus the bf16-free S/d stage. No u stage, no fz stage at all.

Other optimizations:
  - x, z, and all folded weights ship/compute in fp16: same 10-bit mantissa
    as the fp32r (tf32-like) mode, so accuracy is unchanged (~7e-4 measured
    vs the 2e-2 budget) while DMA halves. z rides as an fp16 hi+lo pair
    (exact to fp32) because d needs full input precision. b1 and the v bias
    fold in via rank-1 matmuls against the valid-row mask, which also keeps
    padded columns exactly zero.
  - neg_T uses relu instead of softplus: S entries have std ~89, so the
    log1p(exp(-|S|)) correction inside a 100+-term mean inside a log is
    ~2e-5 relative. This deletes the whole Abs/Exp/Ln/reduce tail over S.
  - One DMA per block (z pair + x k-chunks stacked in one fp16 tensor);
    the cost model charges ~650ns issue + ~625ns HWDGE per DMA, so few
    large transfers win. The last block splits z early / x per k-chunk so
    its v chain clears and h1 accumulates while data streams in.
  - Both slots' v weights batch into one [128, 115] stationary operand
    (slot 1's rows at partition 64: partition starts must be 32-aligned;
    per-slot copies bring them back to base 0 for the S matmuls).
  - The device returns d and sum_j relu(S) per row; the final
    log(softplus(d)+eps) - log(mean+eps) is O(N) float64 work on the host
    during unsharding (exact softplus, no LUT range issues).
  - S matmuls are emitted before the d matmuls so the in-order PE queue
    never waits on the DVE h1*v products mid-stage.
"""

import sys

for _p in ("/opt/trn_rl_repo", "/root/.axon_site/_ro/trn_rl_repo"):
    if _p not in sys.path:
        sys.path.append(_p)

import numpy as np

import concourse.bacc as bacc
import concourse.tile as tile
from concourse import mybir as mb
from concourse.bass_utils import run_bass_kernel_spmd

# ---------------------------------------------------------------- constants
N, IN, Z, C, H = 8192, 512, 128, 64, 50
NCORES = 8
G = C // NCORES          # category slots per core
KX = IN // 128           # k-tiles for x
KZ = 2 + KX              # fp16 row-groups in the xz tensor: zh zl x0..x3
EPS = 1e-8
N_WARM = 8
HB = H + 1               # 51: h1 rows plus the ones row at partition H
VP1 = 64                 # partition base of slot-1's v rows (multiple of 32)

F = mb.ActivationFunctionType
OP = mb.AluOpType
FP32 = mb.dt.float32
FP32R = mb.dt.float32r
FP16 = mb.dt.float16
BF16 = mb.dt.bfloat16

_PROGRAMS = {}


class Layout:
    """Slot/chunk/block geometry baked into the program (shared by cores)."""

    def __init__(self, widths):
        assert len(widths) == G
        self.W = list(widths)
        self.OFF = np.concatenate([[0], np.cumsum(self.W)]).astype(int)
        self.R = int(self.OFF[-1])
        # chunks: (slot, coff, cw, ci)
        self.chunks = []
        for s, w in enumerate(self.W):
            for coff in range(0, w, 128):
                self.chunks.append((s, coff, min(128, w - coff), len(self.chunks)))
        self.NCHUNK = len(self.chunks)
        self.blocks = [(s, min(s + 2, G)) for s in range(0, G, 2)]
        NB = len(self.blocks)
        # packA column layout (fp16; W1/M span partitions, the rest are row-0)
        o = 0
        self.PK_W1 = (o, o + KX * H)
        o += KX * H
        self.PK_M = (o, o + NB * (VP1 + HB))
        o += NB * (VP1 + HB)
        self.PK_B1R = (o, o + H)
        o += H
        self.PK_CB = (o, o + NB * (VP1 + HB))
        o += NB * (VP1 + HB)
        self.PK_MROW = (o, o + self.R)
        self.PW = o + self.R

    def ok(self):
        return all(
            int(self.OFF[s1] - self.OFF[s0]) >= 256 for s0, s1 in self.blocks
        ) and max(self.W) <= 170 and all(s1 - s0 == 2 for s0, s1 in self.blocks)

    def key(self):
        return tuple(self.W)


def _build_program(L: Layout):
    nc = bacc.Bacc("TRN2", target_bir_lowering=False, debug=False)

    R, NC_ = L.R, L.NCHUNK
    d_xz = nc.dram_tensor("xz", [KZ * 128, R], FP16, kind="ExternalInput").ap()
    d_packA = nc.dram_tensor("packA", [128, L.PW], FP16, kind="ExternalInput").ap()
    d_hones = nc.dram_tensor("hones", [1, R], FP16, kind="ExternalInput").ap()
    d_yout = nc.dram_tensor("yout", [128 * 2 * NC_], FP32, kind="ExternalOutput").ap()

    xz_view = d_xz.rearrange("(k p) n -> p k n", p=128)
    NB = len(L.blocks)

    with tile.TileContext(nc) as tc:
        with (
            tc.tile_pool(name="const", bufs=1) as const,
            tc.tile_pool(name="junk", bufs=3) as junkp,
            tc.tile_pool(name="psum_h", bufs=2, space="PSUM") as psum_h,
            tc.tile_pool(name="psum_v", bufs=2, space="PSUM") as psum_v,
            tc.tile_pool(name="psum_s", bufs=2, space="PSUM") as psum_s,
            tc.tile_pool(name="psum_d", bufs=1, space="PSUM") as psum_d,
        ):
            # ---- constants
            s_ones = const.tile([128, 1], FP32)
            nc.vector.memset(s_ones[:], 1.0)
            # the one ACT table set (id 6) holding Copy/Relu used below
            nc.scalar.add_instruction(
                mb.InstLoadActFuncSet(
                    name=nc.get_next_instruction_name(),
                    ins=[],
                    outs=[],
                    act_func_set_id=6,
                )
            )
            s_warmact = const.tile([128, 1], FP32)
            nc.scalar.activation(out=s_warmact[:], in_=s_ones[:], func=F.Abs)

            # ---- persistent tiles
            s_xz = const.tile([128, KZ, R], FP16)
            s_h1T = const.tile([HB, R], FP16)
            s_v16 = const.tile([HB, R], FP16)
            s_out = const.tile([128, 2, NC_], FP32)  # [:,0,:] relu-sums, [:,1,:] d
            s_packA = const.tile([128, L.PW], FP16)

            # ---- all DMAs up front in issue order
            nc.sync.dma_start(out=s_packA[:], in_=d_packA[:])
            nc.sync.dma_start(out=s_h1T[H : H + 1, :], in_=d_hones[:])
            for bi, (s0, s1) in enumerate(L.blocks):
                ns = slice(int(L.OFF[s0]), int(L.OFF[s1]))
                if bi == NB - 1:
                    # z first, then x per k-chunk: the v chain clears early
                    # and h1 accumulates while x streams in
                    nc.sync.dma_start(out=s_xz[:, 0:2, ns], in_=xz_view[:, 0:2, ns])
                    for k in range(KX):
                        nc.sync.dma_start(
                            out=s_xz[:, 2 + k, ns], in_=xz_view[:, 2 + k, ns]
                        )
                else:
                    nc.sync.dma_start(out=s_xz[:, :, ns], in_=xz_view[:, :, ns])

            s_w1 = s_packA[:, L.PK_W1[0] : L.PK_W1[1]].rearrange(
                "p (k h) -> p k h", k=KX
            )
            s_M = s_packA[:, L.PK_M[0] : L.PK_M[1]].rearrange("p (b q) -> p b q", b=NB)
            s_b1r = s_packA[0:1, L.PK_B1R[0] : L.PK_B1R[1]]
            s_cb = s_packA[0:1, L.PK_CB[0] : L.PK_CB[1]].rearrange(
                "p (b q) -> p b q", b=NB
            )
            s_mrow = s_packA[0:1, L.PK_MROW[0] : L.PK_MROW[1]]

            # PE warm-up to start the p-state ramp while DMA runs
            pwarm = psum_v.tile([1, 64], FP32, tag="pv")
            s_wrhs = const.tile([128, 64], FP32)
            nc.vector.memset(s_wrhs[:], 0.0)
            for _ in range(N_WARM):
                nc.tensor.matmul(
                    pwarm[:], lhsT=s_ones[:], rhs=s_wrhs[:], start=True, stop=True
                )

            # chunks narrower than 128 leave tail partitions untouched
            nc.vector.memset(s_out[:], 0.0)
            pd = psum_d.tile([128, NC_], FP32)
            nc.vector.memset(pd[:], 0.0)

            for bi, (s0, s1) in enumerate(L.blocks):
                boff = int(L.OFF[s0])
                bw = int(L.OFF[s1] - L.OFF[s0])
                ns = slice(boff, boff + bw)

                # v = M z + c x mrow for both slots at once: rows 0:51 slot
                # s0, rows 64:115 slot s1 (32-aligned partition bases). The
                # rank-1 bias leads so the z matmuls close the group.
                pv = psum_v.tile([VP1 + HB, bw], FP32, tag="pv")
                nc.tensor.matmul(
                    pv[:], lhsT=s_cb[:, bi, :], rhs=s_mrow[:, ns],
                    start=True, stop=False,
                )
                nc.tensor.matmul(
                    pv[:], lhsT=s_M[:, bi, :], rhs=s_xz[:, 0, ns],
                    start=False, stop=False,
                )
                nc.tensor.matmul(
                    pv[:], lhsT=s_M[:, bi, :], rhs=s_xz[:, 1, ns],
                    start=False, stop=True,
                )
                # per-slot copies shift slot 1's rows down to partition base 0
                # (matmul needs lhsT/rhs bases to match); ACT, since gpsimd
                # cannot touch PSUM on real hardware
                for j, s in enumerate(range(s0, s1)):
                    w = L.W[s]
                    so = int(L.OFF[s]) - boff
                    nc.scalar.activation(
                        out=s_v16[:, boff + so : boff + so + w],
                        in_=pv[VP1 * j : VP1 * j + HB, so : so + w],
                        func=F.Copy,
                    )

                # h1 = relu(W1^T x + b1 x mrow); bias mm first so the last
                # x k-chunk is the only gate on closing the group
                ph = psum_h.tile([H, bw], FP32, tag="ph")
                nc.tensor.matmul(
                    ph[:], lhsT=s_b1r, rhs=s_mrow[:, ns], start=True, stop=False
                )
                for k in range(KX):
                    nc.tensor.matmul(
                        ph[:],
                        lhsT=s_w1[:, k, :],
                        rhs=s_xz[:, 2 + k, ns],
                        start=False,
                        stop=(k == KX - 1),
                    )
                nc.scalar.activation(
                    out=s_h1T[0:H, ns], in_=ph[:], func=F.Relu
                )

                # d products on DVE while the S matmuls run
                s_prodb = junkp.tile([HB, 2, 256], FP32, tag="prod")
                for j, s in enumerate(range(s0, s1)):
                    w = L.W[s]
                    soff = int(L.OFF[s])
                    nc.vector.tensor_mul(
                        s_prodb[:, j, 0:w],
                        s_h1T[0:HB, soff : soff + w],
                        pv[VP1 * j : VP1 * j + HB, soff - boff : soff - boff + w],
                    )

                # S chunks (stride 170: three <=170-col chunks in one bank),
                # then relu row-sums; d matmuls last (they wait on DVE)
                bchunks = [ch for ch in L.chunks if s0 <= ch[0] < s1]
                pS = psum_s.tile([128, len(bchunks), 170], FP32, tag="ps")
                for (cs, coff, cw, ci) in bchunks:
                    soff = int(L.OFF[cs])
                    w = L.W[cs]
                    ck = ci - bchunks[0][3]
                    nc.tensor.matmul(
                        pS[0:cw, ck, 0:w],
                        lhsT=s_h1T[:, soff + coff : soff + coff + cw],
                        rhs=s_v16[:, soff : soff + w],
                        start=True,
                        stop=True,
                    )
                    # sum_j relu(S) straight from PSUM (accum_out's reduction
                    # op is op1 -> must stay add); the 1/n mean folds into the
                    # host-side final
                    jk = junkp.tile([128, 256], FP32, tag="junk")
                    acc = s_out[0:cw, 0, ci : ci + 1]
                    if ck == 2:
                        nc.scalar.activation(
                            out=jk[0:cw, 0:w], in_=pS[0:cw, ck, 0:w],
                            func=F.Relu, accum_out=acc,
                        )
                    else:
                        nc.vector.tensor_scalar(
                            out=jk[0:cw, 0:w], in0=pS[0:cw, ck, 0:w],
                            scalar1=0.0, scalar2=None, op0=OP.max, op1=OP.add,
                            accum_out=acc,
                        )
                for (cs, coff, cw, ci) in bchunks:
                    j = cs - s0
                    nc.tensor.matmul(
                        pd[0:cw, ci : ci + 1],
                        lhsT=s_prodb[:, j, coff : coff + cw],
                        rhs=s_ones[0:HB, :],
                        start=True,
                        stop=True,
                    )

            # d column straight out of PSUM into the output tile
            nc.vector.tensor_copy(s_out[:, 1, :], pd[:])
            nc.sync.dma_start(
                out=d_yout.rearrange("(p t c) -> p t c", p=128, t=2), in_=s_out[:]
            )

    nc.compile()
    return nc


def get_program(L: Layout):
    k = L.key()
    if k not in _PROGRAMS:
        _PROGRAMS[k] = _build_program(L)
    return _PROGRAMS[k]


# ---------------------------------------------------------------- host side
def _assign(cf):
    """Rank-sort categories; rank group g goes to slot position POS[g] so
    adjacent slot pairs (the matmul blocks) are >= 256 wide."""
    sizes = np.array([(cf == k).sum() for k in range(C)])
    order = np.argsort(-sizes, kind="stable")
    pos_of_group = [0, 2, 4, 6, 7, 5, 3, 1]
    widths = [0] * G
    catmap = [[0] * G for _ in range(NCORES)]
    nmap = [[0] * G for _ in range(NCORES)]
    for g in range(G):
        grp = order[8 * g : 8 * g + 8]
        p = pos_of_group[g]
        widths[p] = int(sizes[grp[0]])
        for core in range(NCORES):
            catmap[core][p] = int(grp[core])
            nmap[core][p] = int(sizes[grp[core]])
    return widths, catmap, nmap


def _prep_core_inputs(L, x, z, Ws, W1, b1, W2, b2, Wz, bz, idx_lists, catmap_c, nmap_c):
    xz = np.zeros((KZ * 128, L.R), np.float16)
    mrow = np.zeros(L.R, np.float32)
    for s in range(G):
        idx = idx_lists[catmap_c[s]]
        n = nmap_c[s]
        lo = int(L.OFF[s])
        if n:
            zT = z[idx].T
            zh = zT.astype(np.float16)
            xz[0:128, lo : lo + n] = zh
            xz[128:256, lo : lo + n] = (zT - zh.astype(np.float32)).astype(np.float16)
            xz[256:, lo : lo + n] = x[idx].T.astype(np.float16)
            mrow[lo : lo + n] = 1.0

    NB = len(L.blocks)
    packA = np.zeros((128, L.PW), np.float16)
    packA[:, L.PK_W1[0] : L.PK_W1[1]] = (
        W1.reshape(KX, 128, H).transpose(1, 0, 2).reshape(128, KX * H)
    ).astype(np.float16)
    # v weights: per block [Z, 115] = [M(s0)^T | zeros | M(s1)^T] where
    # M[g] = W2s_aug[g] Wz^T, c[g] = W2s_aug[g] bz, W2s_aug = [W2 Ws; b2 Ws]
    Wz64 = Wz.astype(np.float64)
    Mpk = np.zeros((128, NB, VP1 + HB))
    cpk = np.zeros((1, NB, VP1 + HB))
    for bi, (s0, s1) in enumerate(L.blocks):
        for j, s in enumerate(range(s0, s1)):
            Wsg = Ws[catmap_c[s]].astype(np.float64)
            aug = np.zeros((HB, Z))
            aug[:H] = W2.astype(np.float64) @ Wsg
            aug[H] = b2.astype(np.float64) @ Wsg
            Mpk[:, bi, VP1 * j : VP1 * j + HB] = (aug @ Wz64.T).T
            cpk[0, bi, VP1 * j : VP1 * j + HB] = aug @ bz.astype(np.float64)
    packA[:, L.PK_M[0] : L.PK_M[1]] = Mpk.reshape(128, -1).astype(np.float16)
    packA[0, L.PK_B1R[0] : L.PK_B1R[1]] = b1.astype(np.float16)
    packA[0, L.PK_CB[0] : L.PK_CB[1]] = cpk.reshape(-1).astype(np.float16)
    packA[0, L.PK_MROW[0] : L.PK_MROW[1]] = mrow.astype(np.float16)

    return {"xz": xz, "packA": packA, "hones": mrow.reshape(1, -1).astype(np.float16)}


def _unpack_core_output(L, y, idx_lists, catmap_c, nmap_c, out):
    """y flat [(p t c)] -> rows; final log(softplus(d)+eps)-log(mean+eps) in
    float64 on the host (O(N) unshard-time scalar work)."""
    y = np.asarray(y).reshape(128, 2, L.NCHUNK).astype(np.float64)
    rel = y[:, 0, :]
    d = y[:, 1, :]
    T = np.log1p(np.exp(-np.abs(d))) + np.maximum(d, 0.0)
    logT = np.log(T + EPS)
    for (s, coff, cw, ci) in L.chunks:
        n = nmap_c[s]
        take = min(cw, n - coff)
        if take > 0:
            idx = idx_lists[catmap_c[s]][coff : coff + take]
            out[idx] = logT[0:take, ci] - np.log(rel[0:take, ci] / n + EPS)


def _numpy_fallback(x, c, z, W1, b1, W2, b2, Wz, bz, Ws):
    x64 = x.astype(np.float64)
    fx = np.maximum(x64 @ W1.astype(np.float64) + b1, 0.0) @ W2.astype(
        np.float64
    ) + b2
    fz = z.astype(np.float64) @ Wz.astype(np.float64) + bz
    u = np.einsum("nd,nde->ne", fx, Ws.astype(np.float64)[c])

    def sp(v):
        return np.log1p(np.exp(-np.abs(v))) + np.maximum(v, 0.0)

    T = sp(np.einsum("ne,ne->n", u, fz))
    out = np.empty(N, np.float64)
    for k in range(C):
        idx = np.where(c == k)[0]
        if len(idx) == 0:
            continue
        Sk = sp(u[idx] @ fz[idx].T)
        out[idx] = np.log(T[idx] + EPS) - np.log(Sk.mean(axis=1) + EPS)
    return out.astype(np.float32)


def kernel(x, c, z, W1, b1, W2, b2, Wz, bz, Ws):
    x = np.ascontiguousarray(np.asarray(x), dtype=np.float32)
    z = np.ascontiguousarray(np.asarray(z), dtype=np.float32)
    W1 = np.ascontiguousarray(np.asarray(W1), dtype=np.float32)
    b1 = np.ascontiguousarray(np.asarray(b1), dtype=np.float32)
    W2 = np.ascontiguousarray(np.asarray(W2), dtype=np.float32)
    b2 = np.ascontiguousarray(np.asarray(b2), dtype=np.float32)
    Wz = np.ascontiguousarray(np.asarray(Wz), dtype=np.float32)
    bz = np.ascontiguousarray(np.asarray(bz), dtype=np.float32)
    Ws = np.ascontiguousarray(np.asarray(Ws), dtype=np.float32)
    cf = np.asarray(c).reshape(-1).astype(np.int64)

    idx_lists = [np.where(cf == k)[0] for k in range(C)]
    sizes = [len(i) for i in idx_lists]
    if max(sizes) > 256 or min(sizes) == 0 or len(cf) != N:
        return _numpy_fallback(x, cf, z, W1, b1, W2, b2, Wz, bz, Ws)

    widths, catmap, nmap = _assign(cf)
    L = Layout(widths)
    if not L.ok():
        return _numpy_fallback(x, cf, z, W1, b1, W2, b2, Wz, bz, Ws)

    in_maps = [
        _prep_core_inputs(
            L, x, z, Ws, W1, b1, W2, b2, Wz, bz, idx_lists, catmap[core], nmap[core]
        )
        for core in range(NCORES)
    ]

    nc = get_program(L)
    res = run_bass_kernel_spmd(nc, in_maps, core_ids=list(range(NCORES)))

    out = np.empty(N, np.float32)
    for core in range(NCORES):
        _unpack_core_output(
            L, res.results[core]["yout"], idx_lists, catmap[core], nmap[core], out
        )
    return out


# revision 27
# speedup vs baseline: 1.4088x; 1.0250x over previous
"""Trainium2 Bass kernel for the CPC contrastive loss problem.

Math (reference):
    fx = relu(x @ W1 + b1) @ W2 + b2          [N, Z]
    fz = z @ Wz + bz                          [N, Z]
    u[n] = fx[n] @ Ws[c[n]]                   [N, Z]
    T = softplus(<u, fz>_row)                 [N]
    neg_T[i] = mean_{j: c[j]==c[i]} softplus(<u[i], fz[j]>)
    out = log(T + eps) - log(neg_T + eps)

Structure: rows are grouped by category on the host; each of the 8 cores gets
8 categories, so the NxN S matrix reduces to per-category blocks (64x less
work). Categories are rank-sorted by size; slot s holds same-rank categories
on every core, so the slot widths W[s] (max size in the rank group) bake into
one SPMD program. Slot positions interleave large/small ranks so adjacent
pairs (the processing blocks) are >= 256 columns wide: matmuls below 256
output columns can run at reduced rate.

Key algebra: with the augmented fold W2s_aug[g] = [W2 Ws[g]; b2 Ws[g]] and
h1aug = [relu(x W1 + b1); 1],
    S = h1aug^T v,   d_i = <h1aug_i, v_i>,   v_j = W2s_aug fz_j,
and since fz is consumed ONLY through v, Wz/bz fold in on the host:
    v = (W2s_aug Wz^T) z + (W2s_aug bz) x mrow.
So the device runs just two matmul stages per block (v from z, h1 from x)
plus the bf16-free S/d stage. No u stage, no fz stage at all.

Other optimizations:
  - x, z, and all folded weights ship/compute in fp16: same 10-bit mantissa
    as the fp32r (tf32-like) mode, so accuracy is unchanged (~7e-4 measured
    vs the 2e-2 budget) while DMA halves. z rides as an fp16 hi+lo pair
    (exact to fp32) because d needs full input precision. b1 and the v bias
    fold in via rank-1 matmuls against the valid-row mask, which also keeps
    padded columns exactly zero.
  - neg_T uses relu instead of softplus: S entries have std ~89, so the
    log1p(exp(-|S|)) correction inside a 100+-term mean inside a log is
    ~2e-5 relative. This deletes the whole Abs/Exp/Ln/reduce tail over S.
  - One DMA per block (z pair + x k-chunks stacked in one fp16 tensor);
    the cost model charges ~650ns issue + ~625ns HWDGE per DMA, so few
    large transfers win. The last block splits z early / x per k-chunk so
    its v chain clears and h1 accumulates while data streams in.
  - Both slots' v weights batch into one [128, 115] stationary operand
    (slot 1's rows at partition 64: partition starts must be 32-aligned;
    per-slot copies bring them back to base 0 for the S matmuls).
  - The device returns d and sum_j relu(S) per row; the final
    log(softplus(d)+eps) - log(mean+eps) is O(N) float64 work on the host
    during unsharding (exact softplus, no LUT range issues).
  - S matmuls are emitted before the d matmuls so the in-order PE queue
    never waits on the DVE h1*v products mid-stage.
"""

import sys

for _p in ("/opt/trn_rl_repo", "/root/.axon_site/_ro/trn_rl_repo"):
    if _p not in sys.path:
        sys.path.append(_p)

import numpy as np

import concourse.bacc as bacc
import concourse.tile as tile
from concourse import mybir as mb
from concourse.bass_utils import run_bass_kernel_spmd

# ---------------------------------------------------------------- constants
N, IN, Z, C, H = 8192, 512, 128, 64, 50
NCORES = 8
G = C // NCORES          # category slots per core
KX = IN // 128           # k-tiles for x
KZ = 2 + KX              # fp16 row-groups in the xz tensor: zh zl x0..x3
EPS = 1e-8
N_WARM = 8
HB = H + 1               # 51: h1 rows plus the ones row at partition H
VP1 = 64                 # partition base of slot-1's v rows (multiple of 32)

F = mb.ActivationFunctionType
OP = mb.AluOpType
FP32 = mb.dt.float32
FP32R = mb.dt.float32r
FP16 = mb.dt.float16
BF16 = mb.dt.bfloat16

_PROGRAMS = {}


class Layout:
    """Slot/chunk/block geometry baked into the program (shared by cores)."""

    def __init__(self, widths):
        assert len(widths) == G
        self.W = list(widths)
        self.OFF = np.concatenate([[0], np.cumsum(self.W)]).astype(int)
        self.R = int(self.OFF[-1])
        # chunks: (slot, coff, cw, ci)
        self.chunks = []
        for s, w in enumerate(self.W):
            for coff in range(0, w, 128):
                self.chunks.append((s, coff, min(128, w - coff), len(self.chunks)))
        self.NCHUNK = len(self.chunks)
        self.blocks = [(s, min(s + 2, G)) for s in range(0, G, 2)]
        NB = len(self.blocks)
        # packA column layout (fp16; W1/M span partitions, the rest are row-0)
        o = 0
        self.PK_W1 = (o, o + KX * H)
        o += KX * H
        self.PK_M = (o, o + NB * (VP1 + HB))
        o += NB * (VP1 + HB)
        self.PK_B1R = (o, o + H)
        o += H
        self.PK_CB = (o, o + NB * (VP1 + HB))
        o += NB * (VP1 + HB)
        self.PK_MROW = (o, o + self.R)
        self.PW = o + self.R

    def ok(self):
        return all(
            int(self.OFF[s1] - self.OFF[s0]) >= 256 for s0, s1 in self.blocks
        ) and max(self.W) <= 170 and all(s1 - s0 == 2 for s0, s1 in self.blocks)

    def key(self):
        return tuple(self.W)


def _build_program(L: Layout):
    nc = bacc.Bacc("TRN2", target_bir_lowering=False, debug=False)

    R, NC_ = L.R, L.NCHUNK
    d_xz = nc.dram_tensor("xz", [KZ * 128, R], FP16, kind="ExternalInput").ap()
    d_packA = nc.dram_tensor("packA", [128, L.PW], FP16, kind="ExternalInput").ap()
    d_hones = nc.dram_tensor("hones", [1, R], FP16, kind="ExternalInput").ap()
    d_yout = nc.dram_tensor("yout", [128 * 2 * NC_], FP32, kind="ExternalOutput").ap()

    xz_view = d_xz.rearrange("(k p) n -> p k n", p=128)
    NB = len(L.blocks)

    with tile.TileContext(nc) as tc:
        with (
            tc.tile_pool(name="const", bufs=1) as const,
            tc.tile_pool(name="junk", bufs=3) as junkp,
            tc.tile_pool(name="psum_h", bufs=2, space="PSUM") as psum_h,
            tc.tile_pool(name="psum_v", bufs=2, space="PSUM") as psum_v,
            tc.tile_pool(name="psum_s", bufs=2, space="PSUM") as psum_s,
            tc.tile_pool(name="psum_d", bufs=1, space="PSUM") as psum_d,
        ):
            # ---- constants
            s_ones = const.tile([128, 1], FP32)
            nc.vector.memset(s_ones[:], 1.0)
            # the one ACT table set (id 6) holding Copy/Relu used below
            nc.scalar.add_instruction(
                mb.InstLoadActFuncSet(
                    name=nc.get_next_instruction_name(),
                    ins=[],
                    outs=[],
                    act_func_set_id=6,
                )
            )
            s_warmact = const.tile([128, 1], FP32)
            nc.scalar.activation(out=s_warmact[:], in_=s_ones[:], func=F.Abs)

            # ---- persistent tiles
            s_xz = const.tile([128, KZ, R], FP16)
            s_h1T = const.tile([HB, R], FP16)
            s_v16 = const.tile([HB, R], FP16)
            s_out = const.tile([128, 2, NC_], FP32)  # [:,0,:] relu-sums, [:,1,:] d
            s_packA = const.tile([128, L.PW], FP16)

            # ---- all DMAs up front in issue order
            nc.sync.dma_start(out=s_packA[:], in_=d_packA[:])
            nc.sync.dma_start(out=s_h1T[H : H + 1, :], in_=d_hones[:])
            for bi, (s0, s1) in enumerate(L.blocks):
                ns = slice(int(L.OFF[s0]), int(L.OFF[s1]))
                if bi == NB - 1:
                    # z first, then x per k-chunk: the v chain clears early
                    # and h1 accumulates while x streams in
                    nc.sync.dma_start(out=s_xz[:, 0:2, ns], in_=xz_view[:, 0:2, ns])
                    for k in range(KX):
                        nc.sync.dma_start(
                            out=s_xz[:, 2 + k, ns], in_=xz_view[:, 2 + k, ns]
                        )
                else:
                    nc.sync.dma_start(out=s_xz[:, :, ns], in_=xz_view[:, :, ns])

            s_w1 = s_packA[:, L.PK_W1[0] : L.PK_W1[1]].rearrange(
                "p (k h) -> p k h", k=KX
            )
            s_M = s_packA[:, L.PK_M[0] : L.PK_M[1]].rearrange("p (b q) -> p b q", b=NB)
            s_b1r = s_packA[0:1, L.PK_B1R[0] : L.PK_B1R[1]]
            s_cb = s_packA[0:1, L.PK_CB[0] : L.PK_CB[1]].rearrange(
                "p (b q) -> p b q", b=NB
            )
            s_mrow = s_packA[0:1, L.PK_MROW[0] : L.PK_MROW[1]]

            # PE warm-up to start the p-state ramp while DMA runs
            pwarm = psum_v.tile([1, 64], FP32, tag="pv")
            s_wrhs = const.tile([128, 64], FP32)
            nc.vector.memset(s_wrhs[:], 0.0)
            for _ in range(N_WARM):
                nc.tensor.matmul(
                    pwarm[:], lhsT=s_ones[:], rhs=s_wrhs[:], start=True, stop=True
                )

            # chunks narrower than 128 leave tail partitions untouched
            nc.vector.memset(s_out[:], 0.0)
            pd = psum_d.tile([128, NC_], FP32)
            nc.vector.memset(pd[:], 0.0)

            state = {}

            def emit_tail(bi):
                s0, s1 = L.blocks[bi]
                boff = int(L.OFF[s0])
                pv, pS, bchunks = state[bi]
                s_prodb = junkp.tile([HB, 2, 256], FP32, tag="prod")
                for j, s in enumerate(range(s0, s1)):
                    w = L.W[s]
                    soff = int(L.OFF[s])
                    nc.vector.tensor_mul(
                        s_prodb[:, j, 0:w],
                        s_h1T[0:HB, soff : soff + w],
                        pv[VP1 * j : VP1 * j + HB, soff - boff : soff - boff + w],
                    )
                for (cs, coff, cw, ci) in bchunks:
                    soff = int(L.OFF[cs])
                    w = L.W[cs]
                    ck = ci - bchunks[0][3]
                    # sum_j relu(S) straight from PSUM (accum_out's reduction
                    # op is op1 -> must stay add); the 1/n mean folds into the
                    # host-side final
                    jk = junkp.tile([128, 256], FP32, tag="junk")
                    acc = s_out[0:cw, 0, ci : ci + 1]
                    if ck == 2:
                        nc.scalar.activation(
                            out=jk[0:cw, 0:w], in_=pS[0:cw, ck, 0:w],
                            func=F.Relu, accum_out=acc,
                        )
                    else:
                        nc.vector.tensor_scalar(
                            out=jk[0:cw, 0:w], in0=pS[0:cw, ck, 0:w],
                            scalar1=0.0, scalar2=None, op0=OP.max, op1=OP.add,
                            accum_out=acc,
                        )
                for (cs, coff, cw, ci) in bchunks:
                    j = cs - s0
                    nc.tensor.matmul(
                        pd[0:cw, ci : ci + 1],
                        lhsT=s_prodb[:, j, coff : coff + cw],
                        rhs=s_ones[0:HB, :],
                        start=True,
                        stop=True,
                    )

            for bi, (s0, s1) in enumerate(L.blocks):
                boff = int(L.OFF[s0])
                bw = int(L.OFF[s1] - L.OFF[s0])
                ns = slice(boff, boff + bw)

                # v = M z + c x mrow for both slots at once: rows 0:51 slot
                # s0, rows 64:115 slot s1 (32-aligned partition bases). The
                # rank-1 bias leads so the z matmuls close the group.
                pv = psum_v.tile([VP1 + HB, bw], FP32, tag="pv")
                nc.tensor.matmul(
                    pv[:], lhsT=s_cb[:, bi, :], rhs=s_mrow[:, ns],
                    start=True, stop=False,
                )
                nc.tensor.matmul(
                    pv[:], lhsT=s_M[:, bi, :], rhs=s_xz[:, 0, ns],
                    start=False, stop=False,
                )
                nc.tensor.matmul(
                    pv[:], lhsT=s_M[:, bi, :], rhs=s_xz[:, 1, ns],
                    start=False, stop=True,
                )
                # per-slot copies shift slot 1's rows down to partition base 0
                # (matmul needs lhsT/rhs bases to match); ACT, since gpsimd
                # cannot touch PSUM on real hardware
                for j, s in enumerate(range(s0, s1)):
                    w = L.W[s]
                    so = int(L.OFF[s]) - boff
                    nc.scalar.activation(
                        out=s_v16[:, boff + so : boff + so + w],
                        in_=pv[VP1 * j : VP1 * j + HB, so : so + w],
                        func=F.Copy,
                    )

                # h1 = relu(W1^T x + b1 x mrow); bias mm first so the last
                # x k-chunk is the only gate on closing the group
                ph = psum_h.tile([H, bw], FP32, tag="ph")
                nc.tensor.matmul(
                    ph[:], lhsT=s_b1r, rhs=s_mrow[:, ns], start=True, stop=False
                )
                for k in range(KX):
                    nc.tensor.matmul(
                        ph[:],
                        lhsT=s_w1[:, k, :],
                        rhs=s_xz[:, 2 + k, ns],
                        start=False,
                        stop=(k == KX - 1),
                    )
                nc.scalar.activation(
                    out=s_h1T[0:H, ns], in_=ph[:], func=F.Relu
                )

                # S chunks (stride 170: three <=170-col chunks in one bank)
                bchunks = [ch for ch in L.chunks if s0 <= ch[0] < s1]
                pS = psum_s.tile([128, len(bchunks), 170], FP32, tag="ps")
                for (cs, coff, cw, ci) in bchunks:
                    soff = int(L.OFF[cs])
                    w = L.W[cs]
                    ck = ci - bchunks[0][3]
                    nc.tensor.matmul(
                        pS[0:cw, ck, 0:w],
                        lhsT=s_h1T[:, soff + coff : soff + coff + cw],
                        rhs=s_v16[:, soff : soff + w],
                        start=True,
                        stop=True,
                    )
                state[bi] = (pv, pS, bchunks)
                # terminal ops (d products, relu row-sums, d matmuls) lag one
                # block so they never head-block the next block's enabling
                # work in the in-order engine queues
                if bi > 0:
                    emit_tail(bi - 1)

            emit_tail(NB - 1)
            # d column straight out of PSUM into the output tile
            nc.vector.tensor_copy(s_out[:, 1, :], pd[:])
            nc.sync.dma_start(
                out=d_yout.rearrange("(p t c) -> p t c", p=128, t=2), in_=s_out[:]
            )

    nc.compile()
    return nc


def get_program(L: Layout):
    k = L.key()
    if k not in _PROGRAMS:
        _PROGRAMS[k] = _build_program(L)
    return _PROGRAMS[k]


# ---------------------------------------------------------------- host side
def _assign(cf):
    """Rank-sort categories; rank group g goes to slot position POS[g] so
    adjacent slot pairs (the matmul blocks) are >= 256 wide."""
    sizes = np.array([(cf == k).sum() for k in range(C)])
    order = np.argsort(-sizes, kind="stable")
    pos_of_group = [0, 2, 4, 6, 7, 5, 3, 1]
    widths = [0] * G
    catmap = [[0] * G for _ in range(NCORES)]
    nmap = [[0] * G for _ in range(NCORES)]
    for g in range(G):
        grp = order[8 * g : 8 * g + 8]
        p = pos_of_group[g]
        widths[p] = int(sizes[grp[0]])
        for core in range(NCORES):
            catmap[core][p] = int(grp[core])
            nmap[core][p] = int(sizes[grp[core]])
    return widths, catmap, nmap


def _prep_core_inputs(L, x, z, Ws, W1, b1, W2, b2, Wz, bz, idx_lists, catmap_c, nmap_c):
    xz = np.zeros((KZ * 128, L.R), np.float16)
    mrow = np.zeros(L.R, np.float32)
    for s in range(G):
        idx = idx_lists[catmap_c[s]]
        n = nmap_c[s]
        lo = int(L.OFF[s])
        if n:
            zT = z[idx].T
            zh = zT.astype(np.float16)
            xz[0:128, lo : lo + n] = zh
            xz[128:256, lo : lo + n] = (zT - zh.astype(np.float32)).astype(np.float16)
            xz[256:, lo : lo + n] = x[idx].T.astype(np.float16)
            mrow[lo : lo + n] = 1.0

    NB = len(L.blocks)
    packA = np.zeros((128, L.PW), np.float16)
    packA[:, L.PK_W1[0] : L.PK_W1[1]] = (
        W1.reshape(KX, 128, H).transpose(1, 0, 2).reshape(128, KX * H)
    ).astype(np.float16)
    # v weights: per block [Z, 115] = [M(s0)^T | zeros | M(s1)^T] where
    # M[g] = W2s_aug[g] Wz^T, c[g] = W2s_aug[g] bz, W2s_aug = [W2 Ws; b2 Ws]
    Wz64 = Wz.astype(np.float64)
    Mpk = np.zeros((128, NB, VP1 + HB))
    cpk = np.zeros((1, NB, VP1 + HB))
    for bi, (s0, s1) in enumerate(L.blocks):
        for j, s in enumerate(range(s0, s1)):
            Wsg = Ws[catmap_c[s]].astype(np.float64)
            aug = np.zeros((HB, Z))
            aug[:H] = W2.astype(np.float64) @ Wsg
            aug[H] = b2.astype(np.float64) @ Wsg
            Mpk[:, bi, VP1 * j : VP1 * j + HB] = (aug @ Wz64.T).T
            cpk[0, bi, VP1 * j : VP1 * j + HB] = aug @ bz.astype(np.float64)
    packA[:, L.PK_M[0] : L.PK_M[1]] = Mpk.reshape(128, -1).astype(np.float16)
    packA[0, L.PK_B1R[0] : L.PK_B1R[1]] = b1.astype(np.float16)
    packA[0, L.PK_CB[0] : L.PK_CB[1]] = cpk.reshape(-1).astype(np.float16)
    packA[0, L.PK_MROW[0] : L.PK_MROW[1]] = mrow.astype(np.float16)

    return {"xz": xz, "packA": packA, "hones": mrow.reshape(1, -1).astype(np.float16)}


def _unpack_core_output(L, y, idx_lists, catmap_c, nmap_c, out):
    """y flat [(p t c)] -> rows; final log(softplus(d)+eps)-log(mean+eps) in
    float64 on the host (O(N) unshard-time scalar work)."""
    y = np.asarray(y).reshape(128, 2, L.NCHUNK).astype(np.float64)
    rel = y[:, 0, :]
    d = y[:, 1, :]
    T = np.log1p(np.exp(-np.abs(d))) + np.maximum(d, 0.0)
    logT = np.log(T + EPS)
    for (s, coff, cw, ci) in L.chunks:
        n = nmap_c[s]
        take = min(cw, n - coff)
        if take > 0:
            idx = idx_lists[catmap_c[s]][coff : coff + take]
            out[idx] = logT[0:take, ci] - np.log(rel[0:take, ci] / n + EPS)


def _numpy_fallback(x, c, z, W1, b1, W2, b2, Wz, bz, Ws):
    x64 = x.astype(np.float64)
    fx = np.maximum(x64 @ W1.astype(np.float64) + b1, 0.0) @ W2.astype(
        np.float64
    ) + b2
    fz = z.astype(np.float64) @ Wz.astype(np.float64) + bz
    u = np.einsum("nd,nde->ne", fx, Ws.astype(np.float64)[c])

    def sp(v):
        return np.log1p(np.exp(-np.abs(v))) + np.maximum(v, 0.0)

    T = sp(np.einsum("ne,ne->n", u, fz))
    out = np.empty(N, np.float64)
    for k in range(C):
        idx = np.where(c == k)[0]
        if len(idx) == 0:
            continue
        Sk = sp(u[idx] @ fz[idx].T)
        out[idx] = np.log(T[idx] + EPS) - np.log(Sk.mean(axis=1) + EPS)
    return out.astype(np.float32)


def kernel(x, c, z, W1, b1, W2, b2, Wz, bz, Ws):
    x = np.ascontiguousarray(np.asarray(x), dtype=np.float32)
    z = np.ascontiguousarray(np.asarray(z), dtype=np.float32)
    W1 = np.ascontiguousarray(np.asarray(W1), dtype=np.float32)
    b1 = np.ascontiguousarray(np.asarray(b1), dtype=np.float32)
    W2 = np.ascontiguousarray(np.asarray(W2), dtype=np.float32)
    b2 = np.ascontiguousarray(np.asarray(b2), dtype=np.float32)
    Wz = np.ascontiguousarray(np.asarray(Wz), dtype=np.float32)
    bz = np.ascontiguousarray(np.asarray(bz), dtype=np.float32)
    Ws = np.ascontiguousarray(np.asarray(Ws), dtype=np.float32)
    cf = np.asarray(c).reshape(-1).astype(np.int64)

    idx_lists = [np.where(cf == k)[0] for k in range(C)]
    sizes = [len(i) for i in idx_lists]
    if max(sizes) > 256 or min(sizes) == 0 or len(cf) != N:
        return _numpy_fallback(x, cf, z, W1, b1, W2, b2, Wz, bz, Ws)

    widths, catmap, nmap = _assign(cf)
    L = Layout(widths)
    if not L.ok():
        return _numpy_fallback(x, cf, z, W1, b1, W2, b2, Wz, bz, Ws)

    in_maps = [
        _prep_core_inputs(
            L, x, z, Ws, W1, b1, W2, b2, Wz, bz, idx_lists, catmap[core], nmap[core]
        )
        for core in range(NCORES)
    ]

    nc = get_program(L)
    res = run_bass_kernel_spmd(nc, in_maps, core_ids=list(range(NCORES)))

    out = np.empty(N, np.float32)
    for core in range(NCORES):
        _unpack_core_output(
            L, res.results[core]["yout"], idx_lists, catmap[core], nmap[core], out
        )
    return out
